# revision 22
# baseline (speedup 1.0000x reference)
"""Trainium2 Bass kernel v4 for 2-layer GATv2 (nn_GCNAttn_1494648619259).

Per-dst-slot layout as v3: dst node = SBUF partition, its in-edges along the
free axis (slot k=0 = self loop); host pre-gathers per-edge SOURCE features
(the halo gather).  v4 restructures the device pipeline:

  * e = y_l[s] + y_r[d] is produced DIRECTLY in PSUM by two accumulating
    matmuls per edge-slab (Wl on the edge column + Wr on the segment's self
    column) - the old eatt DVE pass and xr2 path are gone.  The L-columns
    (F:F+H) hold 0.55*(L[s]+R[d]) for the score's linear part.
  * |e| on GpSimd (tensor_scalar abs_max 0) - frees DVE.
  * score contraction sum_c sign_c*|e_c| via a compile-time sign-range
    halving TREE on DVE f16 (<=2 ops/level/head), not 1x tensor_reduce.
  * softmax un-normalized: weights = exp(score - selfscore) * mask (f16);
    den is written out and the division happens on HOST.
  * weighted segment-sum on the PE: identity-lhsT matmuls accumulate
    G = exm (.) e chunks into a PSUM tile per dst-tile; host subtracts
    den*xr (out = sum exm*e - den*xr = sum exm*y_l) and normalizes.

Sharding: 8 cores = 2 graphs x 4 quarters (unchanged from v3).
"""
import numpy as np
from contextlib import ExitStack

import concourse.bass as bass
import concourse.mybir as mybir
import concourse.tile as tile
from concourse import bacc
from concourse.bass_utils import run_bass_kernel_spmd

# ---- problem constants ----
H = 2
C = 64
F = 2 * C            # 128
NEG = 0.1
A_ = (1 + NEG) / 2.0  # 0.55
B_ = (1 - NEG) / 2.0  # 0.45
N = 20000
Bn = 2
F_IN = 32
NT = 160
P = 128
NPAD = NT * P        # 20480
NG = 40              # groups == own tiles per core
NCORES = 8
RW = 132             # matmul out row: 128 y + 2 (0.55*L) + 2 pad
SLAB = 6             # k-cols per convert batch: 2 PSUM banks, 3 cols each
BANK = 512           # PSUM bank, f32 elems
WCAP = 64            # max k-columns (S*kwb) per processing block

_F32 = mybir.dt.float32
_F16 = mybir.dt.float16


# ======================= host-side planning =======================

def _plan(edge_index):
    src = edge_index[0].astype(np.int64)
    dst = edge_index[1].astype(np.int64)
    E = len(src)

    deg = np.bincount(dst, minlength=N)          # in-degree excl self loop
    order = np.argsort(-deg, kind="stable")
    rank_of = np.empty(N, np.int64)
    rank_of[order] = np.arange(N)

    deg_by_rank = np.zeros(NPAD, np.int64)
    deg_by_rank[:N] = deg[order]
    KW = np.zeros(NG, np.int64)
    for j in range(NG):
        KW[j] = deg_by_rank[j * 512:(j + 1) * 512].max() + 1
    KW = ((KW + 3) // 4) * 4                      # multiple of 4

    # blocks: greedily merge adjacent groups while S*kwb <= WCAP
    blocks = []                                   # list of (tiles, KWB)
    j = 0
    while j < NG:
        kwb = int(KW[j])
        S = 1
        while j + S < NG and (S + 1) * max(kwb, int(KW[j + S])) <= WCAP:
            kwb = max(kwb, int(KW[j + S]))
            S += 1
        blocks.append((list(range(j, j + S)), kwb))
        j += S
    boff = []                                     # slot offset per block
    off = 0
    for tiles, kwb in blocks:
        boff.append(off)
        off += len(tiles) * kwb
    SLOTW = off                                   # total k-columns
    SLOT = SLOTW * P

    # node id per rank; dummy ranks (>= N) -> id N (zero feature column)
    ids = np.concatenate([order, np.full(NPAD - N, N, np.int64)])

    rd = rank_of[dst]
    qd = (rd // P) % 4
    jd = rd // 512
    pd = rd % P
    sort_d = np.argsort(rd, kind="stable")
    starts = np.searchsorted(rd[sort_d], rd)
    invpos = np.empty(E, np.int64)
    invpos[sort_d] = np.arange(E)
    kidx = invpos - starts
    assert np.all(kidx + 1 <= KW[jd] - 1)

    # per-group column offset: group j -> block bb, seg s
    jcol = np.zeros(NG, np.int64)                 # k-col offset of group j
    for (tiles, kwb), off in zip(blocks, boff):
        for s, t in enumerate(tiles):
            jcol[t] = off + s * kwb

    srcid = np.zeros((4, SLOT), np.int32)
    dstid = np.zeros((4, SLOT), np.int32)
    mask01 = np.zeros((4, P, SLOTW), np.float16)
    own_ranks = []
    for q in range(4):
        r_all = np.arange(NPAD)
        own = r_all[(r_all // P) % 4 == q]
        own_ranks.append(own)
        sid = np.empty(SLOT, np.int64)
        mq = np.zeros((P, SLOTW), np.float16)
        for (tiles, kwb), off in zip(blocks, boff):
            for s, t in enumerate(tiles):
                ranks = np.arange(512 * t + 128 * q, 512 * t + 128 * (q + 1))
                dst_ids = ids[ranks]
                co = off + s * kwb
                sid[co * P:(co + kwb) * P] = np.tile(dst_ids, kwb)
                dslot = deg_by_rank[ranks]
                karr = np.arange(kwb)[None, :]
                mq[:, co:co + kwb] = (karr <= dslot[:, None]).astype(
                    np.float16)
        dstid[q] = sid.astype(np.int32)          # pre-scatter: dst id per slot
        sel = qd == q
        col = (jcol[jd[sel]] + kidx[sel] + 1) * P + pd[sel]
        sid[col] = src[sel]
        srcid[q] = sid.astype(np.int32)
        mask01[q] = mq

    return dict(order=order, rank_of=rank_of, KW=KW, blocks=blocks,
                boff=boff, SLOTW=SLOTW, SLOT=SLOT, srcid=srcid, dstid=dstid,
                mask01=mask01, own_ranks=own_ranks)


def _layer_consts(Wl, bl, Wr, br, att, bias):
    att = np.asarray(att, np.float64)
    perm = np.concatenate([
        h * C + np.concatenate([np.nonzero(att[h] >= 0)[0],
                                np.nonzero(att[h] < 0)[0]])
        for h in range(H)]).astype(np.int64)
    npos = np.array([(att[h] >= 0).sum() for h in range(H)], np.int64)
    attp = att.reshape(-1)[perm]
    aab = np.abs(attp)
    Wl = np.asarray(Wl, np.float64)[:, perm]
    Wr = np.asarray(Wr, np.float64)[:, perm]
    bl = np.asarray(bl, np.float64)[perm]
    br = np.asarray(br, np.float64)[perm]
    K = Wl.shape[0]
    has_bias = bool(np.any(bl != 0) or np.any(br != 0))
    if has_bias:
        Kx = K + 1
        assert Kx <= P, "K=128 with nonzero table bias unsupported"
    else:
        Kx = K
    # stacked mode: [fg_src; fg_dst] with [Wl; Wr] -> one matmul per k-col
    stacked = 2 * Kx <= P
    Wl_ext = np.zeros((Kx, RW), np.float32)
    Wr_ext = np.zeros((Kx, RW), np.float32)
    Wl_ext[:K, :F] = Wl * aab[None, :]
    Wr_ext[:K, :F] = Wr * aab[None, :]
    if Kx > K:
        Wl_ext[K, :F] = bl * aab
        Wr_ext[K, :F] = br * aab
    for h in range(H):
        sl = slice(h * C, (h + 1) * C)
        Wl_ext[:K, F + h] = A_ * (Wl[:, sl] @ attp[sl])
        Wr_ext[:K, F + h] = A_ * (Wr[:, sl] @ attp[sl])
        if Kx > K:
            Wl_ext[K, F + h] = A_ * (bl[sl] @ attp[sl])
            Wr_ext[K, F + h] = A_ * (br[sl] @ attp[sl])
    return dict(perm=perm, npos=npos, Kx=Kx, stacked=stacked,
                Wl_ext=Wl_ext.astype(np.float16),
                Wr_ext=Wr_ext.astype(np.float16),
                inv=(1.0 / aab).astype(np.float32),
                bias_p=np.asarray(bias, np.float32)[perm])


def _tree_schedule(p0):
    """Halving-tree ops for one head's 64 channels, tag-monotone.

    Returns (levels, sigma): levels = list of op-lists; each op is
    (i0, i1, sub) - combine cur[i0:i1] with cur[i0+w:i1+w] via sub?lo-hi:lo+hi
    producing out[i0:i1].  Tags stay [+^a, -^(w-a)] with a = min(a, w).
    sigma = final +-1 sign of the single output value.
    """
    a = int(p0)
    w = 64
    levels = []
    while w > 1:
        w //= 2
        ops = []
        hi_a = max(0, min(a - w, w))   # (+,+) range [0, hi_a)
        mid = min(a, w)                # (+,-) range [hi_a, mid)
        if hi_a > 0:
            ops.append((0, hi_a, False))
        if mid > hi_a:
            ops.append((hi_a, mid, True))
        if w > mid:
            ops.append((mid, w, False))   # (-,-) -> add, tag -1
        levels.append(ops)
        a = min(a, w)
    sigma = 1.0 if a >= 1 else -1.0
    return levels, sigma


# ======================= bass program =======================

def _build_layer(nc, Kx, plan, npos, stacked):
    blocks = plan["blocks"]
    boff = plan["boff"]
    SLOTW = plan["SLOTW"]
    SLOT = plan["SLOT"]
    Kf = 2 * Kx if stacked else Kx              # fg partition rows

    fg_in = nc.dram_tensor("fg", [Kf, SLOT], _F16, kind="ExternalInput").ap()
    Wl_in = nc.dram_tensor("Wl_ext", [Kf, RW], _F16, kind="ExternalInput").ap()
    if not stacked:
        Wr_in = nc.dram_tensor("Wr_ext", [Kx, RW], _F16,
                               kind="ExternalInput").ap()
    mask_in = nc.dram_tensor("mask01", [P, SLOTW], _F16,
                             kind="ExternalInput").ap()
    id_in = nc.dram_tensor("ident", [P, P], _F16, kind="ExternalInput").ap()
    hout = nc.dram_tensor("hout", [NG * P, F], _F32, kind="ExternalOutput").ap()
    dend = nc.dram_tensor("dend", [NG * P, H], _F32, kind="ExternalOutput").ap()

    p0, p1 = int(npos[0]), int(npos[1])
    tree_h = [_tree_schedule(p0), _tree_schedule(p1)]

    with tile.TileContext(nc) as tc, ExitStack() as ctx:
        const = ctx.enter_context(tc.tile_pool(name="const", bufs=1))
        fgp = ctx.enter_context(tc.tile_pool(name="fgp", bufs=2))
        psp = ctx.enter_context(tc.tile_pool(name="psp", bufs=3, space="PSUM"))
        ps4p = ctx.enter_context(tc.tile_pool(name="ps4p", bufs=2,
                                              space="PSUM"))
        esp = ctx.enter_context(tc.tile_pool(name="esp", bufs=2))
        abp = ctx.enter_context(tc.tile_pool(name="abp", bufs=2))
        trp = ctx.enter_context(tc.tile_pool(name="trp", bufs=1))
        gp = ctx.enter_context(tc.tile_pool(name="gp", bufs=2))
        sml = ctx.enter_context(tc.tile_pool(name="sml", bufs=2))
        outp = ctx.enter_context(tc.tile_pool(name="outp", bufs=2))

        wl_sb = const.tile([Kf, RW], _F16)
        nc.sync.dma_start(wl_sb[:], Wl_in[:])
        if not stacked:
            wr_sb = const.tile([Kx, RW], _F16)
            nc.sync.dma_start(wr_sb[:], Wr_in[:])
        mask_sb = const.tile([P, SLOTW], _F16)
        nc.sync.dma_start(mask_sb[:], mask_in[:])
        id_sb = const.tile([P, P], _F16)
        nc.sync.dma_start(id_sb[:], id_in[:])

        def production(bb):
            """fg DMA + accumulating e-matmuls + PSUM->f16 converts."""
            tiles, kwb = blocks[bb]
            off = boff[bb]
            S = len(tiles)
            W = S * kwb
            fgt = fgp.tile([Kf, WCAP * P], _F16, tag="fg")
            nc.sync.dma_start(fgt[:, :W * P], fg_in[:, off * P:(off + W) * P])
            eS = esp.tile([P, WCAP, RW], _F16, tag="eS")
            nb = (W + SLAB - 1) // SLAB
            for b in range(nb):
                k0 = b * SLAB
                ns = min(SLAB, W - k0)
                ps = psp.tile([P, 2 * BANK], _F32, tag="ps", space="PSUM")
                for s in range(ns):
                    k = k0 + s
                    po = (s // 3) * BANK + (s % 3) * RW
                    if stacked:
                        nc.tensor.matmul(
                            ps[:, po:po + RW],
                            fgt[:, k * P:(k + 1) * P], wl_sb[:],
                            start=True, stop=True, skip_group_check=True)
                    else:
                        sc_col = (k // kwb) * kwb    # segment self column
                        nc.tensor.matmul(
                            ps[:, po:po + RW],
                            fgt[:, k * P:(k + 1) * P], wl_sb[:],
                            start=True, stop=False, skip_group_check=True)
                        nc.tensor.matmul(
                            ps[:, po:po + RW],
                            fgt[:, sc_col * P:(sc_col + 1) * P], wr_sb[:],
                            start=False, stop=True, skip_group_check=True)
                if ns == SLAB:
                    nc.scalar.activation(
                        eS[:, k0:k0 + ns, :].rearrange(
                            "p (b k) r -> p b (k r)", b=2),
                        ps[:].rearrange(
                            "p (b c) -> p b c", b=2)[:, :, :3 * RW],
                        mybir.ActivationFunctionType.Copy)
                else:
                    for b0 in range(0, ns, 3):
                        n0 = min(3, ns - b0)
                        nc.scalar.activation(
                            eS[:, k0 + b0:k0 + b0 + n0, :].rearrange(
                                "p k r -> p (k r)"),
                            ps[:, (b0 // 3) * BANK:
                               (b0 // 3) * BANK + n0 * RW],
                            mybir.ActivationFunctionType.Copy)
            return (tiles, kwb, S, W, off, eS)

        def consume(prod):
            tiles, kwb, S, W, off, eS = prod
            eS_r = eS[:, :W, :].rearrange("p (s k) r -> p s k r", s=S)

            # |e| split: first half ACT Abs, second half DVE abs_max (4x)
            AB = abp.tile([P, WCAP, F], _F16, tag="AB")
            half = (W + 1) // 2
            if half > 0:
                nc.scalar.activation(
                    AB[:, 0:half, :], eS[:, 0:half, 0:F],
                    mybir.ActivationFunctionType.Abs)
            if W > half:
                # |x| = max(-x, x); abs_max is not a legal TT/TS alu op
                nc.vector.scalar_tensor_tensor(
                    out=AB[:, half:W, :], in0=eS[:, half:W, 0:F],
                    scalar=-1.0, in1=eS[:, half:W, 0:F],
                    op0=mybir.AluOpType.mult, op1=mybir.AluOpType.max)

            # sign-range halving tree per head -> T1 [P, W, 2]
            tiers = {}
            cur_w = 32
            Tprev = None
            # level widths: 32,16,8,4,2,1; tile holds both heads side by side
            for li in range(6):
                w = 32 >> li
                T = trp.tile([P, WCAP, 2 * w], _F16, tag=f"T{w}")
                for h in range(H):
                    ops = tree_h[h][0][li]
                    if li == 0:
                        src = AB[:, :W, h * 64:(h + 1) * 64]
                    else:
                        pw = 2 * w
                        src = Tprev[:, :W, h * pw:(h + 1) * pw]
                    dst = T[:, :W, h * w:(h + 1) * w]
                    for (i0, i1, sub) in ops:
                        nc.vector.tensor_tensor(
                            out=dst[:, :, i0:i1],
                            in0=src[:, :, i0:i1],
                            in1=src[:, :, w + i0:w + i1],
                            op=(mybir.AluOpType.subtract if sub
                                else mybir.AluOpType.add))
                Tprev = T
            T1 = Tprev   # [P, W(cap), 2]
            T1_r = T1[:, :W, :].rearrange("p (s k) t -> p t s k", s=S)

            # sc[h] = (sigma_h*B_)*T1[h] + 0.55*(L+R)  (col F+h of eS)
            sc = sml.tile([P, H, S, WCAP], _F32, tag="sc")
            for h in range(H):
                sig = tree_h[h][1]
                nc.vector.scalar_tensor_tensor(
                    out=sc[:, h, :, :kwb], in0=T1_r[:, h],
                    scalar=float(B_ * sig),
                    in1=eS_r[:, :, :, F + h],
                    op0=mybir.AluOpType.mult, op1=mybir.AluOpType.add)

            # shift by the self slot's score (softmax-invariant; keeps exp
            # and the un-normalized weights in f16 range), then one exp
            sc2 = sml.tile([P, H, S, WCAP], _F32, tag="sc2")
            nc.vector.tensor_tensor(
                out=sc2[:, :, :, :kwb], in0=sc[:, :, :, :kwb],
                in1=sc[:, :, :, 0:1].to_broadcast([P, H, S, kwb]),
                op=mybir.AluOpType.subtract)
            exm = sml.tile([P, H, S, WCAP], _F16, tag="exm")
            nc.scalar.activation(
                exm[:, :, :, :kwb], sc2[:, :, :, :kwb],
                mybir.ActivationFunctionType.Exp)

            # mask
            exmm = sml.tile([P, H, S, WCAP], _F16, tag="exmm")
            mvv = mask_sb[:, off:off + W].rearrange(
                "p (s k) -> p s k", s=S).unsqueeze(1).to_broadcast(
                [P, H, S, kwb])
            nc.vector.tensor_tensor(
                out=exmm[:, :, :, :kwb], in0=exm[:, :, :, :kwb], in1=mvv,
                op=mybir.AluOpType.mult)

            # den + one DMA per block ([P, S, H] layout matches dend rows)
            den = sml.tile([P, S, H], _F32, tag="den")
            nc.vector.tensor_reduce(
                out=den[:, :, :].rearrange("p s h -> p h s"),
                in_=exmm[:, :, :, :kwb],
                axis=mybir.AxisListType.X, op=mybir.AluOpType.add)
            t0 = tiles[0]
            nc.sync.dma_start(
                dend[t0 * P:(t0 + S) * P, :].rearrange(
                    "(s p) h -> p s h", p=P),
                den[:, :S, :])

            # expand exmm to [P, W, C] per head on gpsimd (1-input, line-rate)
            # so the G multiply runs in DVE 2x mode (broadcast in1 forces 1x)
            G = gp.tile([P, WCAP, F], _F16, tag="G")
            for h in range(H):
                EX = gp.tile([P, WCAP, C], _F16, tag="EX")
                EX_r = EX[:, :W, :].rearrange("p (s k) c -> p s k c", s=S)
                nc.gpsimd.tensor_copy(
                    EX_r[:], exmm[:, h, :, :kwb].unsqueeze(3).to_broadcast(
                        [P, S, kwb, C]))
                nc.vector.tensor_tensor(
                    out=G[:, :W, h * C:(h + 1) * C],
                    in0=eS[:, :W, h * C:(h + 1) * C], in1=EX[:, :W, :],
                    op=mybir.AluOpType.mult)

            # weighted segment-sum on PE: identity-lhsT accumulation
            ngrp = (S + 3) // 4
            for g in range(ngrp):
                s0 = g * 4
                nseg = min(4, S - s0)
                ps4 = ps4p.tile([P, 4 * P], _F32, tag="ps4", space="PSUM")
                for si in range(nseg):
                    s = s0 + si
                    for k in range(kwb):
                        nc.tensor.matmul(
                            ps4[:, si * P:(si + 1) * P],
                            id_sb[:], G[:, s * kwb + k, 0:F],
                            start=(k == 0), stop=(k == kwb - 1),
                            skip_group_check=True)
                hsb = outp.tile([P, 4 * P], _F32, tag="hsb")
                nc.vector.tensor_copy(hsb[:, :nseg * P], ps4[:, :nseg * P])
                t0 = tiles[s0]
                nc.sync.dma_start(
                    hout[t0 * P:(t0 + nseg) * P, :].rearrange(
                        "(s p) c -> p s c", p=P),
                    hsb[:, :nseg * P].rearrange("p (s c) -> p s c", s=nseg))

        NB = len(blocks)
        prod = production(0)
        for bb in range(NB):
            cur = prod
            if bb + 1 < NB:
                prod = production(bb + 1)
            consume(cur)
    return nc


def _compile_layer(Kx, plan, npos, stacked):
    nc = bacc.Bacc("TRN2", target_bir_lowering=False, debug=False,
                   enable_asserts=False, num_devices=NCORES,
                   num_swdge_queues=1)
    _build_layer(nc, Kx, plan, npos, stacked)
    nc.compile()
    return nc


# ======================= top-level =======================

def _core_inputs(plan, lc, feats):
    """feats: [Bn] arrays [N, K] float32 (node-id space, std channels)."""
    K = feats[0].shape[1]
    Kx = lc["Kx"]
    stacked = lc["stacked"]
    maps = []
    ftp = []
    for g in range(Bn):
        fp = np.zeros((Kx, N + 1), np.float16)
        fp[:K, :N] = feats[g].T
        if Kx > K:
            fp[K, :N] = 1.0
        ftp.append(fp)
    ident = np.eye(P, dtype=np.float16)
    for core in range(NCORES):
        g, q = core // 4, core % 4
        fg = ftp[g][:, plan["srcid"][q]]
        if stacked:
            fg = np.vstack([fg, ftp[g][:, plan["dstid"][q]]])
            wl = np.vstack([lc["Wl_ext"], lc["Wr_ext"]])
            maps.append({
                "fg": np.ascontiguousarray(fg),
                "Wl_ext": np.ascontiguousarray(wl),
                "mask01": plan["mask01"][q],
                "ident": ident,
            })
        else:
            maps.append({
                "fg": np.ascontiguousarray(fg),
                "Wl_ext": lc["Wl_ext"], "Wr_ext": lc["Wr_ext"],
                "mask01": plan["mask01"][q],
                "ident": ident,
            })
    return maps, ftp


_RESULTS_LOG = {}


def _gather_h(plan, res, lc, ftp):
    """Host: h = (hout/den - xr) * inv + bias, un-permuted."""
    perm_inv = np.empty(F, np.int64)
    perm_inv[lc["perm"]] = np.arange(F)
    # device-matching xr per graph: [N, RW-first-F]
    xr_h = [ftp[g].astype(np.float32).T @ lc["Wr_ext"].astype(np.float32)
            for g in range(Bn)]
    h = np.zeros((Bn, N, F), np.float32)
    for core in range(NCORES):
        g, q = core // 4, core % 4
        rows = res.results[core]["hout"].astype(np.float32)
        dens = res.results[core]["dend"].astype(np.float32)
        own = plan["own_ranks"][q]
        real = own < N
        ids = plan["order"][own[real]]
        r = rows[real]
        d = dens[real]
        dexp = np.repeat(d, C, axis=1)                 # [n, F]
        h[g, ids] = r / dexp - xr_h[g][ids][:, :F]
    h = h * lc["inv"][None, None, :] + lc["bias_p"][None, None, :]
    return h[:, :, perm_inv]


def kernel(x, edge_index, Wl1, bl1, Wr1, br1, att1, bias1,
           Wl2, bl2, Wr2, br2, att2, bias2):
    x = np.asarray(x, np.float32)
    edge_index = np.asarray(edge_index)
    plan = _plan(edge_index)
    lc1 = _layer_consts(Wl1, bl1, Wr1, br1, att1, bias1)
    lc2 = _layer_consts(Wl2, bl2, Wr2, br2, att2, bias2)

    feats1 = [np.ascontiguousarray(x[g].T) for g in range(Bn)]
    nc1 = _compile_layer(lc1["Kx"], plan, lc1["npos"], lc1["stacked"])
    maps1, ftp1 = _core_inputs(plan, lc1, feats1)
    res1 = run_bass_kernel_spmd(nc1, maps1, list(range(NCORES)))
    _RESULTS_LOG["l1"] = res1
    h1 = _gather_h(plan, res1, lc1, ftp1)

    feats2 = [np.ascontiguousarray(h1[g]) for g in range(Bn)]
    nc2 = _compile_layer(lc2["Kx"], plan, lc2["npos"], lc2["stacked"])
    maps2, ftp2 = _core_inputs(plan, lc2, feats2)
    res2 = run_bass_kernel_spmd(nc2, maps2, list(range(NCORES)))
    _RESULTS_LOG["l2"] = res2
    h2 = _gather_h(plan, res2, lc2, ftp2)

    return np.ascontiguousarray(np.transpose(h2, (0, 2, 1)))


# revision 26
# speedup vs baseline: 1.7584x; 1.7584x over previous
"""Trainium2 Bass kernel v4 for 2-layer GATv2 (nn_GCNAttn_1494648619259).

Per-dst-slot layout as v3: dst node = SBUF partition, its in-edges along the
free axis (slot k=0 = self loop); host pre-gathers per-edge SOURCE features
(the halo gather).  v4 restructures the device pipeline:

  * e = y_l[s] + y_r[d] is produced DIRECTLY in PSUM by two accumulating
    matmuls per edge-slab (Wl on the edge column + Wr on the segment's self
    column) - the old eatt DVE pass and xr2 path are gone.  The L-columns
    (F:F+H) hold 0.55*(L[s]+R[d]) for the score's linear part.
  * |e| on GpSimd (tensor_scalar abs_max 0) - frees DVE.
  * score contraction sum_c sign_c*|e_c| via a compile-time sign-range
    halving TREE on DVE f16 (<=2 ops/level/head), not 1x tensor_reduce.
  * softmax un-normalized: weights = exp(score - selfscore) * mask (f16);
    den is written out and the division happens on HOST.
  * weighted segment-sum on the PE: identity-lhsT matmuls accumulate
    G = exm (.) e chunks into a PSUM tile per dst-tile; host subtracts
    den*xr (out = sum exm*e - den*xr = sum exm*y_l) and normalizes.

Sharding: 8 cores = 2 graphs x 4 quarters (unchanged from v3).
"""
import numpy as np
from contextlib import ExitStack

import concourse.bass as bass
import concourse.mybir as mybir
import concourse.tile as tile
from concourse import bacc
from concourse.bass_utils import run_bass_kernel_spmd

# ---- problem constants ----
H = 2
C = 64
F = 2 * C            # 128
NEG = 0.1
A_ = (1 + NEG) / 2.0  # 0.55
B_ = (1 - NEG) / 2.0  # 0.45
N = 20000
Bn = 2
F_IN = 32
NT = 160
P = 128
NPAD = NT * P        # 20480
NG = 40              # groups == own tiles per core
NCORES = 8
RW = 132             # matmul out row: 128 y + 2 (0.55*L) + 2 pad
SLAB = 6             # k-cols per convert batch: 2 PSUM banks, 3 cols each
BANK = 512           # PSUM bank, f32 elems
WCAP = 64            # max k-columns (S*kwb) per processing block

_F32 = mybir.dt.float32
_F16 = mybir.dt.float16


# ======================= host-side planning =======================

def _plan(edge_index):
    src = edge_index[0].astype(np.int64)
    dst = edge_index[1].astype(np.int64)
    E = len(src)

    deg = np.bincount(dst, minlength=N)          # in-degree excl self loop
    order = np.argsort(-deg, kind="stable")
    rank_of = np.empty(N, np.int64)
    rank_of[order] = np.arange(N)

    deg_by_rank = np.zeros(NPAD, np.int64)
    deg_by_rank[:N] = deg[order]
    KW = np.zeros(NG, np.int64)
    for j in range(NG):
        KW[j] = deg_by_rank[j * 512:(j + 1) * 512].max() + 1
    KW = ((KW + 3) // 4) * 4                      # multiple of 4

    # blocks: greedily merge adjacent groups while S*kwb <= WCAP
    blocks = []                                   # list of (tiles, KWB)
    j = 0
    while j < NG:
        kwb = int(KW[j])
        S = 1
        while j + S < NG and (S + 1) * max(kwb, int(KW[j + S])) <= WCAP:
            kwb = max(kwb, int(KW[j + S]))
            S += 1
        blocks.append((list(range(j, j + S)), kwb))
        j += S
    boff = []                                     # slot offset per block
    off = 0
    for tiles, kwb in blocks:
        boff.append(off)
        off += len(tiles) * kwb
    SLOTW = off                                   # total k-columns
    SLOT = SLOTW * P

    # node id per rank; dummy ranks (>= N) -> id N (zero feature column)
    ids = np.concatenate([order, np.full(NPAD - N, N, np.int64)])

    rd = rank_of[dst]
    qd = (rd // P) % 4
    jd = rd // 512
    pd = rd % P
    sort_d = np.argsort(rd, kind="stable")
    starts = np.searchsorted(rd[sort_d], rd)
    invpos = np.empty(E, np.int64)
    invpos[sort_d] = np.arange(E)
    kidx = invpos - starts
    assert np.all(kidx + 1 <= KW[jd] - 1)

    # per-group column offset: group j -> block bb, seg s
    jcol = np.zeros(NG, np.int64)                 # k-col offset of group j
    for (tiles, kwb), off in zip(blocks, boff):
        for s, t in enumerate(tiles):
            jcol[t] = off + s * kwb

    srcid = np.zeros((4, SLOT), np.int32)
    dstid = np.zeros((4, SLOT), np.int32)
    mask01 = np.zeros((4, P, SLOTW), np.float16)
    own_ranks = []
    for q in range(4):
        r_all = np.arange(NPAD)
        own = r_all[(r_all // P) % 4 == q]
        own_ranks.append(own)
        sid = np.empty(SLOT, np.int64)
        mq = np.zeros((P, SLOTW), np.float16)
        for (tiles, kwb), off in zip(blocks, boff):
            for s, t in enumerate(tiles):
                ranks = np.arange(512 * t + 128 * q, 512 * t + 128 * (q + 1))
                dst_ids = ids[ranks]
                co = off + s * kwb
                sid[co * P:(co + kwb) * P] = np.tile(dst_ids, kwb)
                dslot = deg_by_rank[ranks]
                karr = np.arange(kwb)[None, :]
                mq[:, co:co + kwb] = (karr <= dslot[:, None]).astype(
                    np.float16)
        dstid[q] = sid.astype(np.int32)          # pre-scatter: dst id per slot
        sel = qd == q
        col = (jcol[jd[sel]] + kidx[sel] + 1) * P + pd[sel]
        sid[col] = src[sel]
        srcid[q] = sid.astype(np.int32)
        mask01[q] = mq

    return dict(order=order, rank_of=rank_of, KW=KW, blocks=blocks,
                boff=boff, SLOTW=SLOTW, SLOT=SLOT, srcid=srcid, dstid=dstid,
                mask01=mask01, own_ranks=own_ranks)


def _layer_consts(Wl, bl, Wr, br, att, bias):
    att = np.asarray(att, np.float64)
    perm = np.concatenate([
        h * C + np.concatenate([np.nonzero(att[h] >= 0)[0],
                                np.nonzero(att[h] < 0)[0]])
        for h in range(H)]).astype(np.int64)
    npos = np.array([(att[h] >= 0).sum() for h in range(H)], np.int64)
    attp = att.reshape(-1)[perm]
    aab = np.abs(attp)
    Wl = np.asarray(Wl, np.float64)[:, perm]
    Wr = np.asarray(Wr, np.float64)[:, perm]
    bl = np.asarray(bl, np.float64)[perm]
    br = np.asarray(br, np.float64)[perm]
    K = Wl.shape[0]
    has_bias = bool(np.any(bl != 0) or np.any(br != 0))
    if has_bias:
        Kx = K + 1
        assert Kx <= P, "K=128 with nonzero table bias unsupported"
    else:
        Kx = K
    # stacked mode: [fg_src; fg_dst] with [Wl; Wr] -> one matmul per k-col
    stacked = 2 * Kx <= P
    Wl_ext = np.zeros((Kx, RW), np.float32)
    Wr_ext = np.zeros((Kx, RW), np.float32)
    Wl_ext[:K, :F] = Wl * aab[None, :]
    Wr_ext[:K, :F] = Wr * aab[None, :]
    if Kx > K:
        Wl_ext[K, :F] = bl * aab
        Wr_ext[K, :F] = br * aab
    for h in range(H):
        sl = slice(h * C, (h + 1) * C)
        Wl_ext[:K, F + h] = A_ * (Wl[:, sl] @ attp[sl])
        Wr_ext[:K, F + h] = A_ * (Wr[:, sl] @ attp[sl])
        if Kx > K:
            Wl_ext[K, F + h] = A_ * (bl[sl] @ attp[sl])
            Wr_ext[K, F + h] = A_ * (br[sl] @ attp[sl])
    return dict(perm=perm, npos=npos, Kx=Kx, stacked=stacked,
                Wl_ext=Wl_ext.astype(np.float16),
                Wr_ext=Wr_ext.astype(np.float16),
                inv=(1.0 / aab).astype(np.float32),
                bias_p=np.asarray(bias, np.float32)[perm])


def _tree_schedule(p0):
    """Halving-tree ops for one head's 64 channels, tag-monotone.

    Returns (levels, sigma): levels = list of op-lists; each op is
    (i0, i1, sub) - combine cur[i0:i1] with cur[i0+w:i1+w] via sub?lo-hi:lo+hi
    producing out[i0:i1].  Tags stay [+^a, -^(w-a)] with a = min(a, w).
    sigma = final +-1 sign of the single output value.
    """
    a = int(p0)
    w = 64
    levels = []
    while w > 1:
        w //= 2
        ops = []
        hi_a = max(0, min(a - w, w))   # (+,+) range [0, hi_a)
        mid = min(a, w)                # (+,-) range [hi_a, mid)
        if hi_a > 0:
            ops.append((0, hi_a, False))
        if mid > hi_a:
            ops.append((hi_a, mid, True))
        if w > mid:
            ops.append((mid, w, False))   # (-,-) -> add, tag -1
        levels.append(ops)
        a = min(a, w)
    sigma = 1.0 if a >= 1 else -1.0
    return levels, sigma


# ======================= bass program =======================

def _build_layer(nc, Kx, plan, npos, stacked):
    blocks = plan["blocks"]
    boff = plan["boff"]
    SLOTW = plan["SLOTW"]
    SLOT = plan["SLOT"]
    Kf = 2 * Kx if stacked else Kx              # fg partition rows

    fg_in = nc.dram_tensor("fg", [Kf, SLOT], _F16, kind="ExternalInput").ap()
    Wl_in = nc.dram_tensor("Wl_ext", [Kf, RW], _F16, kind="ExternalInput").ap()
    if not stacked:
        Wr_in = nc.dram_tensor("Wr_ext", [Kx, RW], _F16,
                               kind="ExternalInput").ap()
    mask_in = nc.dram_tensor("mask01", [P, SLOTW], _F16,
                             kind="ExternalInput").ap()
    id_in = nc.dram_tensor("ident", [P, P], _F16, kind="ExternalInput").ap()
    hout = nc.dram_tensor("hout", [NG * P, F], _F32, kind="ExternalOutput").ap()
    dend = nc.dram_tensor("dend", [NG * P, H], _F32, kind="ExternalOutput").ap()

    p0, p1 = int(npos[0]), int(npos[1])
    tree_h = [_tree_schedule(p0), _tree_schedule(p1)]

    with tile.TileContext(nc) as tc, ExitStack() as ctx:
        const = ctx.enter_context(tc.tile_pool(name="const", bufs=1))
        fgp = ctx.enter_context(tc.tile_pool(name="fgp", bufs=2))
        psp = ctx.enter_context(tc.tile_pool(name="psp", bufs=3, space="PSUM"))
        ps4p = ctx.enter_context(tc.tile_pool(name="ps4p", bufs=2,
                                              space="PSUM"))
        esp = ctx.enter_context(tc.tile_pool(name="esp", bufs=2))
        abp = ctx.enter_context(tc.tile_pool(name="abp", bufs=2))
        trp = ctx.enter_context(tc.tile_pool(name="trp", bufs=2))
        gp = ctx.enter_context(tc.tile_pool(name="gp", bufs=2))
        sml = ctx.enter_context(tc.tile_pool(name="sml", bufs=2))
        outp = ctx.enter_context(tc.tile_pool(name="outp", bufs=2))

        wl_sb = const.tile([Kf, RW], _F16)
        nc.sync.dma_start(wl_sb[:], Wl_in[:])
        if not stacked:
            wr_sb = const.tile([Kx, RW], _F16)
            nc.sync.dma_start(wr_sb[:], Wr_in[:])
        mask_sb = const.tile([P, SLOTW], _F16)
        nc.sync.dma_start(mask_sb[:], mask_in[:])
        id_sb = const.tile([P, P], _F16)
        nc.sync.dma_start(id_sb[:], id_in[:])

        def production(bb):
            """fg DMA + accumulating e-matmuls + PSUM->f16 converts."""
            tiles, kwb = blocks[bb]
            off = boff[bb]
            S = len(tiles)
            W = S * kwb
            fgt = fgp.tile([Kf, WCAP * P], _F16, tag="fg")
            nc.sync.dma_start(fgt[:, :W * P], fg_in[:, off * P:(off + W) * P])
            eS = esp.tile([P, WCAP, RW], _F16, tag="eS")
            nb = (W + SLAB - 1) // SLAB
            for b in range(nb):
                k0 = b * SLAB
                ns = min(SLAB, W - k0)
                ps = psp.tile([P, 2 * BANK], _F32, tag="ps", space="PSUM")
                for s in range(ns):
                    k = k0 + s
                    po = (s // 3) * BANK + (s % 3) * RW
                    if stacked:
                        nc.tensor.matmul(
                            ps[:, po:po + RW],
                            fgt[:, k * P:(k + 1) * P], wl_sb[:],
                            start=True, stop=True, skip_group_check=True)
                    else:
                        sc_col = (k // kwb) * kwb    # segment self column
                        nc.tensor.matmul(
                            ps[:, po:po + RW],
                            fgt[:, k * P:(k + 1) * P], wl_sb[:],
                            start=True, stop=False, skip_group_check=True)
                        nc.tensor.matmul(
                            ps[:, po:po + RW],
                            fgt[:, sc_col * P:(sc_col + 1) * P], wr_sb[:],
                            start=False, stop=True, skip_group_check=True)
                if ns == SLAB:
                    nc.scalar.activation(
                        eS[:, k0:k0 + ns, :].rearrange(
                            "p (b k) r -> p b (k r)", b=2),
                        ps[:].rearrange(
                            "p (b c) -> p b c", b=2)[:, :, :3 * RW],
                        mybir.ActivationFunctionType.Copy)
                else:
                    for b0 in range(0, ns, 3):
                        n0 = min(3, ns - b0)
                        nc.scalar.activation(
                            eS[:, k0 + b0:k0 + b0 + n0, :].rearrange(
                                "p k r -> p (k r)"),
                            ps[:, (b0 // 3) * BANK:
                               (b0 // 3) * BANK + n0 * RW],
                            mybir.ActivationFunctionType.Copy)
            return (tiles, kwb, S, W, off, eS)

        def consume(prod):
            tiles, kwb, S, W, off, eS = prod
            eS_r = eS[:, :W, :].rearrange("p (s k) r -> p s k r", s=S)

            # |e| split: first half ACT Abs, second half DVE abs_max (4x)
            AB = abp.tile([P, WCAP, F], _F16, tag="AB")
            half = (W + 1) // 2
            if half > 0:
                nc.scalar.activation(
                    AB[:, 0:half, :], eS[:, 0:half, 0:F],
                    mybir.ActivationFunctionType.Abs)
            if W > half:
                # |x| = max(-x, x); abs_max is not a legal TT/TS alu op
                nc.vector.scalar_tensor_tensor(
                    out=AB[:, half:W, :], in0=eS[:, half:W, 0:F],
                    scalar=-1.0, in1=eS[:, half:W, 0:F],
                    op0=mybir.AluOpType.mult, op1=mybir.AluOpType.max)

            # sign-range halving tree per head -> T1 [P, W, 2]
            tiers = {}
            cur_w = 32
            Tprev = None
            # level widths: 32,16,8,4,2,1; tile holds both heads side by side
            for li in range(6):
                w = 32 >> li
                T = trp.tile([P, WCAP, 2 * w], _F16, tag=f"T{w}")
                for h in range(H):
                    ops = tree_h[h][0][li]
                    if li == 0:
                        src = AB[:, :W, h * 64:(h + 1) * 64]
                    else:
                        pw = 2 * w
                        src = Tprev[:, :W, h * pw:(h + 1) * pw]
                    dst = T[:, :W, h * w:(h + 1) * w]
                    for (i0, i1, sub) in ops:
                        nc.vector.tensor_tensor(
                            out=dst[:, :, i0:i1],
                            in0=src[:, :, i0:i1],
                            in1=src[:, :, w + i0:w + i1],
                            op=(mybir.AluOpType.subtract if sub
                                else mybir.AluOpType.add))
                Tprev = T
            T1 = Tprev   # [P, W(cap), 2]
            T1_r = T1[:, :W, :].rearrange("p (s k) t -> p t s k", s=S)

            # sc[h] = (sigma_h*B_)*T1[h] + 0.55*(L+R)  (col F+h of eS)
            sc = sml.tile([P, H, S, WCAP], _F32, tag="sc")
            for h in range(H):
                sig = tree_h[h][1]
                nc.vector.scalar_tensor_tensor(
                    out=sc[:, h, :, :kwb], in0=T1_r[:, h],
                    scalar=float(B_ * sig),
                    in1=eS_r[:, :, :, F + h],
                    op0=mybir.AluOpType.mult, op1=mybir.AluOpType.add)

            # shift by the self slot's score (softmax-invariant; keeps the
            # un-normalized f16 weights in range): exp bias per (h, s)
            selfneg = sml.tile([P, H, S], _F32, tag="sn")
            nc.scalar.activation(
                selfneg[:], sc[:, :, :, 0],
                mybir.ActivationFunctionType.Copy, scale=-1.0)
            exm = sml.tile([P, H, S, WCAP], _F16, tag="exm")
            for h in range(H):
                for s in range(S):
                    nc.scalar.activation(
                        exm[:, h, s, :kwb], sc[:, h, s, :kwb],
                        mybir.ActivationFunctionType.Exp,
                        bias=selfneg[:, h, s:s + 1])

            # mask
            exmm = sml.tile([P, H, S, WCAP], _F16, tag="exmm")
            mvv = mask_sb[:, off:off + W].rearrange(
                "p (s k) -> p s k", s=S).unsqueeze(1).to_broadcast(
                [P, H, S, kwb])
            nc.vector.tensor_tensor(
                out=exmm[:, :, :, :kwb], in0=exm[:, :, :, :kwb], in1=mvv,
                op=mybir.AluOpType.mult)

            # den + DMA per segment
            den = sml.tile([P, H, S], _F32, tag="den")
            nc.vector.tensor_reduce(
                out=den[:], in_=exmm[:, :, :, :kwb],
                axis=mybir.AxisListType.X, op=mybir.AluOpType.add)
            for s, t in enumerate(tiles):
                nc.sync.dma_start(dend[t * P:(t + 1) * P, :], den[:, :, s])

            # G = exm (.) e   (per head, broadcast over 64 channels)
            G = gp.tile([P, WCAP, F], _F16, tag="G")
            G_r = G[:, :W, :].rearrange("p (s k) f -> p s k f", s=S)
            for h in range(H):
                nc.vector.tensor_tensor(
                    out=G_r[:, :, :, h * C:(h + 1) * C],
                    in0=eS_r[:, :, :, h * C:(h + 1) * C],
                    in1=exmm[:, h, :, :kwb].unsqueeze(3).to_broadcast(
                        [P, S, kwb, C]),
                    op=mybir.AluOpType.mult)

            # weighted segment-sum on PE: identity-lhsT accumulation
            ngrp = (S + 3) // 4
            for g in range(ngrp):
                s0 = g * 4
                nseg = min(4, S - s0)
                ps4 = ps4p.tile([P, 4 * P], _F32, tag="ps4", space="PSUM")
                for si in range(nseg):
                    s = s0 + si
                    for k in range(kwb):
                        nc.tensor.matmul(
                            ps4[:, si * P:(si + 1) * P],
                            id_sb[:], G[:, s * kwb + k, 0:F],
                            start=(k == 0), stop=(k == kwb - 1),
                            skip_group_check=True)
                hsb = outp.tile([P, 4 * P], _F32, tag="hsb")
                nc.vector.tensor_copy(hsb[:, :nseg * P], ps4[:, :nseg * P])
                t0 = tiles[s0]
                nc.sync.dma_start(
                    hout[t0 * P:(t0 + nseg) * P, :].rearrange(
                        "(s p) c -> p s c", p=P),
                    hsb[:, :nseg * P].rearrange("p (s c) -> p s c", s=nseg))

        NB = len(blocks)
        prod = production(0)
        for bb in range(NB):
            cur = prod
            if bb + 1 < NB:
                prod = production(bb + 1)
            consume(cur)
    return nc


def _compile_layer(Kx, plan, npos, stacked):
    nc = bacc.Bacc("TRN2", target_bir_lowering=False, debug=False,
                   enable_asserts=False, num_devices=NCORES,
                   num_swdge_queues=1)
    _build_layer(nc, Kx, plan, npos, stacked)
    nc.compile()
    return nc


# ======================= top-level =======================

def _core_inputs(plan, lc, feats):
    """feats: [Bn] arrays [N, K] float32 (node-id space, std channels)."""
    K = feats[0].shape[1]
    Kx = lc["Kx"]
    stacked = lc["stacked"]
    maps = []
    ftp = []
    for g in range(Bn):
        fp = np.zeros((Kx, N + 1), np.float16)
        fp[:K, :N] = feats[g].T
        if Kx > K:
            fp[K, :N] = 1.0
        ftp.append(fp)
    ident = np.eye(P, dtype=np.float16)
    for core in range(NCORES):
        g, q = core // 4, core % 4
        fg = ftp[g][:, plan["srcid"][q]]
        if stacked:
            fg = np.vstack([fg, ftp[g][:, plan["dstid"][q]]])
            wl = np.vstack([lc["Wl_ext"], lc["Wr_ext"]])
            maps.append({
                "fg": np.ascontiguousarray(fg),
                "Wl_ext": np.ascontiguousarray(wl),
                "mask01": plan["mask01"][q],
                "ident": ident,
            })
        else:
            maps.append({
                "fg": np.ascontiguousarray(fg),
                "Wl_ext": lc["Wl_ext"], "Wr_ext": lc["Wr_ext"],
                "mask01": plan["mask01"][q],
                "ident": ident,
            })
    return maps, ftp


_RESULTS_LOG = {}


def _gather_h(plan, res, lc, ftp):
    """Host: h = (hout/den - xr) * inv + bias, un-permuted."""
    perm_inv = np.empty(F, np.int64)
    perm_inv[lc["perm"]] = np.arange(F)
    # device-matching xr per graph: [N, RW-first-F]
    xr_h = [ftp[g].astype(np.float32).T @ lc["Wr_ext"].astype(np.float32)
            for g in range(Bn)]
    h = np.zeros((Bn, N, F), np.float32)
    for core in range(NCORES):
        g, q = core // 4, core % 4
        rows = res.results[core]["hout"].astype(np.float32)
        dens = res.results[core]["dend"].astype(np.float32)
        own = plan["own_ranks"][q]
        real = own < N
        ids = plan["order"][own[real]]
        r = rows[real]
        d = dens[real]
        dexp = np.repeat(d, C, axis=1)                 # [n, F]
        h[g, ids] = r / dexp - xr_h[g][ids][:, :F]
    h = h * lc["inv"][None, None, :] + lc["bias_p"][None, None, :]
    return h[:, :, perm_inv]


def kernel(x, edge_index, Wl1, bl1, Wr1, br1, att1, bias1,
           Wl2, bl2, Wr2, br2, att2, bias2):
    x = np.asarray(x, np.float32)
    edge_index = np.asarray(edge_index)
    plan = _plan(edge_index)
    lc1 = _layer_consts(Wl1, bl1, Wr1, br1, att1, bias1)
    lc2 = _layer_consts(Wl2, bl2, Wr2, br2, att2, bias2)

    feats1 = [np.ascontiguousarray(x[g].T) for g in range(Bn)]
    nc1 = _compile_layer(lc1["Kx"], plan, lc1["npos"], lc1["stacked"])
    maps1, ftp1 = _core_inputs(plan, lc1, feats1)
    res1 = run_bass_kernel_spmd(nc1, maps1, list(range(NCORES)))
    _RESULTS_LOG["l1"] = res1
    h1 = _gather_h(plan, res1, lc1, ftp1)

    feats2 = [np.ascontiguousarray(h1[g]) for g in range(Bn)]
    nc2 = _compile_layer(lc2["Kx"], plan, lc2["npos"], lc2["stacked"])
    maps2, ftp2 = _core_inputs(plan, lc2, feats2)
    res2 = run_bass_kernel_spmd(nc2, maps2, list(range(NCORES)))
    _RESULTS_LOG["l2"] = res2
    h2 = _gather_h(plan, res2, lc2, ftp2)

    return np.ascontiguousarray(np.transpose(h2, (0, 2, 1)))


# revision 31
# speedup vs baseline: 2.0343x; 1.1569x over previous
"""Trainium2 Bass kernel v4 for 2-layer GATv2 (nn_GCNAttn_1494648619259).

Per-dst-slot layout as v3: dst node = SBUF partition, its in-edges along the
free axis (slot k=0 = self loop); host pre-gathers per-edge SOURCE features
(the halo gather).  v4 restructures the device pipeline:

  * e = y_l[s] + y_r[d] is produced DIRECTLY in PSUM by two accumulating
    matmuls per edge-slab (Wl on the edge column + Wr on the segment's self
    column) - the old eatt DVE pass and xr2 path are gone.  The L-columns
    (F:F+H) hold 0.55*(L[s]+R[d]) for the score's linear part.
  * |e| on GpSimd (tensor_scalar abs_max 0) - frees DVE.
  * score contraction sum_c sign_c*|e_c| via a compile-time sign-range
    halving TREE on DVE f16 (<=2 ops/level/head), not 1x tensor_reduce.
  * softmax un-normalized: weights = exp(score - selfscore) * mask (f16);
    den is written out and the division happens on HOST.
  * weighted segment-sum on the PE: identity-lhsT matmuls accumulate
    G = exm (.) e chunks into a PSUM tile per dst-tile; host subtracts
    den*xr (out = sum exm*e - den*xr = sum exm*y_l) and normalizes.

Sharding: 8 cores = 2 graphs x 4 quarters (unchanged from v3).
"""
import numpy as np
from contextlib import ExitStack

import concourse.bass as bass
import concourse.mybir as mybir
import concourse.tile as tile
from concourse import bacc
from concourse.bass_utils import run_bass_kernel_spmd

# ---- problem constants ----
H = 2
C = 64
F = 2 * C            # 128
NEG = 0.1
A_ = (1 + NEG) / 2.0  # 0.55
B_ = (1 - NEG) / 2.0  # 0.45
N = 20000
Bn = 2
F_IN = 32
NT = 160
P = 128
NPAD = NT * P        # 20480
NG = 40              # groups == own tiles per core
NCORES = 8
RW = 132             # matmul out row: 128 y + 2 (0.55*L) + 2 pad
SLAB = 6             # k-cols per convert batch: 2 PSUM banks, 3 cols each
BANK = 512           # PSUM bank, f32 elems
WCAP = 64            # max k-columns (S*kwb) per processing block

_F32 = mybir.dt.float32
_F16 = mybir.dt.float16


# ======================= host-side planning =======================

def _plan(edge_index):
    src = edge_index[0].astype(np.int64)
    dst = edge_index[1].astype(np.int64)
    E = len(src)

    deg = np.bincount(dst, minlength=N)          # in-degree excl self loop
    order = np.argsort(-deg, kind="stable")
    rank_of = np.empty(N, np.int64)
    rank_of[order] = np.arange(N)

    deg_by_rank = np.zeros(NPAD, np.int64)
    deg_by_rank[:N] = deg[order]
    KW = np.zeros(NG, np.int64)
    for j in range(NG):
        KW[j] = deg_by_rank[j * 512:(j + 1) * 512].max() + 1
    KW = ((KW + 3) // 4) * 4                      # multiple of 4

    # blocks: greedily merge adjacent groups while S*kwb <= WCAP
    blocks = []                                   # list of (tiles, KWB)
    j = 0
    while j < NG:
        kwb = int(KW[j])
        S = 1
        while j + S < NG and (S + 1) * max(kwb, int(KW[j + S])) <= WCAP:
            kwb = max(kwb, int(KW[j + S]))
            S += 1
        blocks.append((list(range(j, j + S)), kwb))
        j += S
    boff = []                                     # slot offset per block
    off = 0
    for tiles, kwb in blocks:
        boff.append(off)
        off += len(tiles) * kwb
    SLOTW = off                                   # total k-columns
    SLOT = SLOTW * P

    # node id per rank; dummy ranks (>= N) -> id N (zero feature column)
    ids = np.concatenate([order, np.full(NPAD - N, N, np.int64)])

    rd = rank_of[dst]
    qd = (rd // P) % 4
    jd = rd // 512
    pd = rd % P
    sort_d = np.argsort(rd, kind="stable")
    starts = np.searchsorted(rd[sort_d], rd)
    invpos = np.empty(E, np.int64)
    invpos[sort_d] = np.arange(E)
    kidx = invpos - starts
    assert np.all(kidx + 1 <= KW[jd] - 1)

    # per-group column offset: group j -> block bb, seg s
    jcol = np.zeros(NG, np.int64)                 # k-col offset of group j
    for (tiles, kwb), off in zip(blocks, boff):
        for s, t in enumerate(tiles):
            jcol[t] = off + s * kwb

    srcid = np.zeros((4, SLOT), np.int32)
    dstid = np.zeros((4, SLOT), np.int32)
    mask01 = np.zeros((4, P, SLOTW), np.float16)
    own_ranks = []
    for q in range(4):
        r_all = np.arange(NPAD)
        own = r_all[(r_all // P) % 4 == q]
        own_ranks.append(own)
        sid = np.empty(SLOT, np.int64)
        mq = np.zeros((P, SLOTW), np.float16)
        for (tiles, kwb), off in zip(blocks, boff):
            for s, t in enumerate(tiles):
                ranks = np.arange(512 * t + 128 * q, 512 * t + 128 * (q + 1))
                dst_ids = ids[ranks]
                co = off + s * kwb
                sid[co * P:(co + kwb) * P] = np.tile(dst_ids, kwb)
                dslot = deg_by_rank[ranks]
                karr = np.arange(kwb)[None, :]
                mq[:, co:co + kwb] = (karr <= dslot[:, None]).astype(
                    np.float16)
        dstid[q] = sid.astype(np.int32)          # pre-scatter: dst id per slot
        sel = qd == q
        col = (jcol[jd[sel]] + kidx[sel] + 1) * P + pd[sel]
        sid[col] = src[sel]
        srcid[q] = sid.astype(np.int32)
        mask01[q] = mq

    return dict(order=order, rank_of=rank_of, KW=KW, blocks=blocks,
                boff=boff, SLOTW=SLOTW, SLOT=SLOT, srcid=srcid, dstid=dstid,
                mask01=mask01, own_ranks=own_ranks)


def _layer_consts(Wl, bl, Wr, br, att, bias):
    att = np.asarray(att, np.float64)
    perm = np.concatenate([
        h * C + np.concatenate([np.nonzero(att[h] >= 0)[0],
                                np.nonzero(att[h] < 0)[0]])
        for h in range(H)]).astype(np.int64)
    npos = np.array([(att[h] >= 0).sum() for h in range(H)], np.int64)
    attp = att.reshape(-1)[perm]
    aab = np.abs(attp)
    Wl = np.asarray(Wl, np.float64)[:, perm]
    Wr = np.asarray(Wr, np.float64)[:, perm]
    bl = np.asarray(bl, np.float64)[perm]
    br = np.asarray(br, np.float64)[perm]
    K = Wl.shape[0]
    has_bias = bool(np.any(bl != 0) or np.any(br != 0))
    if has_bias:
        Kx = K + 1
        assert Kx <= P, "K=128 with nonzero table bias unsupported"
    else:
        Kx = K
    # stacked mode: [fg_src; fg_dst] with [Wl; Wr] -> one matmul per k-col
    stacked = 2 * Kx <= P
    Wl_ext = np.zeros((Kx, RW), np.float32)
    Wr_ext = np.zeros((Kx, RW), np.float32)
    Wl_ext[:K, :F] = Wl * aab[None, :]
    Wr_ext[:K, :F] = Wr * aab[None, :]
    if Kx > K:
        Wl_ext[K, :F] = bl * aab
        Wr_ext[K, :F] = br * aab
    for h in range(H):
        sl = slice(h * C, (h + 1) * C)
        Wl_ext[:K, F + h] = A_ * (Wl[:, sl] @ attp[sl])
        Wr_ext[:K, F + h] = A_ * (Wr[:, sl] @ attp[sl])
        if Kx > K:
            Wl_ext[K, F + h] = A_ * (bl[sl] @ attp[sl])
            Wr_ext[K, F + h] = A_ * (br[sl] @ attp[sl])
    return dict(perm=perm, npos=npos, Kx=Kx, stacked=stacked,
                Wl_ext=Wl_ext.astype(np.float16),
                Wr_ext=Wr_ext.astype(np.float16),
                inv=(1.0 / aab).astype(np.float32),
                bias_p=np.asarray(bias, np.float32)[perm])


def _tree_schedule(p0):
    """Halving-tree ops for one head's 64 channels, tag-monotone.

    Returns (levels, sigma): levels = list of op-lists; each op is
    (i0, i1, sub) - combine cur[i0:i1] with cur[i0+w:i1+w] via sub?lo-hi:lo+hi
    producing out[i0:i1].  Tags stay [+^a, -^(w-a)] with a = min(a, w).
    sigma = final +-1 sign of the single output value.
    """
    a = int(p0)
    w = 64
    levels = []
    while w > 1:
        w //= 2
        ops = []
        hi_a = max(0, min(a - w, w))   # (+,+) range [0, hi_a)
        mid = min(a, w)                # (+,-) range [hi_a, mid)
        if hi_a > 0:
            ops.append((0, hi_a, False))
        if mid > hi_a:
            ops.append((hi_a, mid, True))
        if w > mid:
            ops.append((mid, w, False))   # (-,-) -> add, tag -1
        levels.append(ops)
        a = min(a, w)
    sigma = 1.0 if a >= 1 else -1.0
    return levels, sigma


# ======================= bass program =======================

def _build_layer(nc, Kx, plan, npos, stacked):
    blocks = plan["blocks"]
    boff = plan["boff"]
    SLOTW = plan["SLOTW"]
    SLOT = plan["SLOT"]
    Kf = 2 * Kx if stacked else Kx              # fg partition rows

    fg_in = nc.dram_tensor("fg", [Kf, SLOT], _F16, kind="ExternalInput").ap()
    Wl_in = nc.dram_tensor("Wl_ext", [Kf, RW], _F16, kind="ExternalInput").ap()
    if not stacked:
        Wr_in = nc.dram_tensor("Wr_ext", [Kx, RW], _F16,
                               kind="ExternalInput").ap()
    mask_in = nc.dram_tensor("mask01", [P, SLOTW], _F16,
                             kind="ExternalInput").ap()
    id_in = nc.dram_tensor("ident", [P, P], _F16, kind="ExternalInput").ap()
    hout = nc.dram_tensor("hout", [NG * P, F], _F32, kind="ExternalOutput").ap()
    dend = nc.dram_tensor("dend", [NG * P, H], _F32, kind="ExternalOutput").ap()

    p0, p1 = int(npos[0]), int(npos[1])
    tree_h = [_tree_schedule(p0), _tree_schedule(p1)]

    with tile.TileContext(nc) as tc, ExitStack() as ctx:
        const = ctx.enter_context(tc.tile_pool(name="const", bufs=1))
        fgp = ctx.enter_context(tc.tile_pool(name="fgp", bufs=2))
        psp = ctx.enter_context(tc.tile_pool(name="psp", bufs=3, space="PSUM"))
        ps4p = ctx.enter_context(tc.tile_pool(name="ps4p", bufs=2,
                                              space="PSUM"))
        esp = ctx.enter_context(tc.tile_pool(name="esp", bufs=2))
        abp = ctx.enter_context(tc.tile_pool(name="abp", bufs=2))
        trp = ctx.enter_context(tc.tile_pool(name="trp", bufs=2))
        gp = ctx.enter_context(tc.tile_pool(name="gp", bufs=2))
        sml = ctx.enter_context(tc.tile_pool(name="sml", bufs=2))
        outp = ctx.enter_context(tc.tile_pool(name="outp", bufs=2))

        wl_sb = const.tile([Kf, RW], _F16)
        nc.sync.dma_start(wl_sb[:], Wl_in[:])
        if not stacked:
            wr_sb = const.tile([Kx, RW], _F16)
            nc.sync.dma_start(wr_sb[:], Wr_in[:])
        mask_sb = const.tile([P, SLOTW], _F16)
        nc.sync.dma_start(mask_sb[:], mask_in[:])
        id_sb = const.tile([P, P], _F16)
        nc.sync.dma_start(id_sb[:], id_in[:])

        def production(bb):
            """fg DMA + accumulating e-matmuls + PSUM->f16 converts."""
            tiles, kwb = blocks[bb]
            off = boff[bb]
            S = len(tiles)
            W = S * kwb
            fgt = fgp.tile([Kf, WCAP * P], _F16, tag="fg")
            nc.sync.dma_start(fgt[:, :W * P], fg_in[:, off * P:(off + W) * P])
            eS = esp.tile([P, WCAP, RW], _F16, tag="eS")
            nb = (W + SLAB - 1) // SLAB
            for b in range(nb):
                k0 = b * SLAB
                ns = min(SLAB, W - k0)
                ps = psp.tile([P, 2 * BANK], _F32, tag="ps", space="PSUM")
                for s in range(ns):
                    k = k0 + s
                    po = (s // 3) * BANK + (s % 3) * RW
                    if stacked:
                        nc.tensor.matmul(
                            ps[:, po:po + RW],
                            fgt[:, k * P:(k + 1) * P], wl_sb[:],
                            start=True, stop=True, skip_group_check=True)
                    else:
                        sc_col = (k // kwb) * kwb    # segment self column
                        nc.tensor.matmul(
                            ps[:, po:po + RW],
                            fgt[:, k * P:(k + 1) * P], wl_sb[:],
                            start=True, stop=False, skip_group_check=True)
                        nc.tensor.matmul(
                            ps[:, po:po + RW],
                            fgt[:, sc_col * P:(sc_col + 1) * P], wr_sb[:],
                            start=False, stop=True, skip_group_check=True)
                if ns == SLAB:
                    nc.scalar.activation(
                        eS[:, k0:k0 + ns, :].rearrange(
                            "p (b k) r -> p b (k r)", b=2),
                        ps[:].rearrange(
                            "p (b c) -> p b c", b=2)[:, :, :3 * RW],
                        mybir.ActivationFunctionType.Copy)
                else:
                    for b0 in range(0, ns, 3):
                        n0 = min(3, ns - b0)
                        nc.scalar.activation(
                            eS[:, k0 + b0:k0 + b0 + n0, :].rearrange(
                                "p k r -> p (k r)"),
                            ps[:, (b0 // 3) * BANK:
                               (b0 // 3) * BANK + n0 * RW],
                            mybir.ActivationFunctionType.Copy)
            return (tiles, kwb, S, W, off, eS)

        def consume_abs(prod):
            """|e| ops: only need this block's eS - emitted BEFORE the next
            block's production so they sit ahead of its converts in the ACT
            FIFO (convoy breaker)."""
            tiles, kwb, S, W, off, eS = prod
            # split by measured rates: ~60% ACT Abs, 40% DVE max(-x, x)
            AB = abp.tile([P, WCAP, F], _F16, tag="AB")
            cut = (3 * W // 5 + 1) & ~1
            cut = min(cut, W)
            if cut > 0:
                nc.scalar.activation(
                    AB[:, 0:cut, :], eS[:, 0:cut, 0:F],
                    mybir.ActivationFunctionType.Abs)
            if W > cut:
                # |x| = max(-x, x); abs_max is not a legal TT/TS alu op
                nc.vector.scalar_tensor_tensor(
                    out=AB[:, cut:W, :], in0=eS[:, cut:W, 0:F],
                    scalar=-1.0, in1=eS[:, cut:W, 0:F],
                    op0=mybir.AluOpType.mult, op1=mybir.AluOpType.max)
            return AB

        def consume(prod, AB):
            tiles, kwb, S, W, off, eS = prod
            eS_r = eS[:, :W, :].rearrange("p (s k) r -> p s k r", s=S)

            # sign-range halving tree per head -> T1 [P, W, 2]
            tiers = {}
            cur_w = 32
            Tprev = None
            # level widths: 32,16,8,4,2,1; tile holds both heads side by side
            for li in range(6):
                w = 32 >> li
                T = trp.tile([P, WCAP, 2 * w], _F16, tag=f"T{w}")
                for h in range(H):
                    ops = tree_h[h][0][li]
                    if li == 0:
                        src = AB[:, :W, h * 64:(h + 1) * 64]
                    else:
                        pw = 2 * w
                        src = Tprev[:, :W, h * pw:(h + 1) * pw]
                    dst = T[:, :W, h * w:(h + 1) * w]
                    for (i0, i1, sub) in ops:
                        nc.vector.tensor_tensor(
                            out=dst[:, :, i0:i1],
                            in0=src[:, :, i0:i1],
                            in1=src[:, :, w + i0:w + i1],
                            op=(mybir.AluOpType.subtract if sub
                                else mybir.AluOpType.add))
                Tprev = T
            T1 = Tprev   # [P, W(cap), 2]
            T1_r = T1[:, :W, :].rearrange("p (s k) t -> p t s k", s=S)

            # sc[h] = (sigma_h*B_)*T1[h] + 0.55*(L+R)  (col F+h of eS)
            sc = sml.tile([P, H, S, WCAP], _F32, tag="sc")
            for h in range(H):
                sig = tree_h[h][1]
                nc.vector.scalar_tensor_tensor(
                    out=sc[:, h, :, :kwb], in0=T1_r[:, h],
                    scalar=float(B_ * sig),
                    in1=eS_r[:, :, :, F + h],
                    op0=mybir.AluOpType.mult, op1=mybir.AluOpType.add)

            # shift by the self slot's score (softmax-invariant; keeps the
            # un-normalized f16 weights in range): exp bias per (h, s)
            selfneg = sml.tile([P, H, S], _F32, tag="sn")
            nc.scalar.activation(
                selfneg[:], sc[:, :, :, 0],
                mybir.ActivationFunctionType.Copy, scale=-1.0)
            exm = sml.tile([P, H, S, WCAP], _F16, tag="exm")
            for h in range(H):
                for s in range(S):
                    nc.scalar.activation(
                        exm[:, h, s, :kwb], sc[:, h, s, :kwb],
                        mybir.ActivationFunctionType.Exp,
                        bias=selfneg[:, h, s:s + 1])

            # mask
            exmm = sml.tile([P, H, S, WCAP], _F16, tag="exmm")
            mvv = mask_sb[:, off:off + W].rearrange(
                "p (s k) -> p s k", s=S).unsqueeze(1).to_broadcast(
                [P, H, S, kwb])
            nc.vector.tensor_tensor(
                out=exmm[:, :, :, :kwb], in0=exm[:, :, :, :kwb], in1=mvv,
                op=mybir.AluOpType.mult)

            # den + DMA per segment
            den = sml.tile([P, H, S], _F32, tag="den")
            nc.vector.tensor_reduce(
                out=den[:], in_=exmm[:, :, :, :kwb],
                axis=mybir.AxisListType.X, op=mybir.AluOpType.add)
            for s, t in enumerate(tiles):
                nc.sync.dma_start(dend[t * P:(t + 1) * P, :], den[:, :, s])

            # G = exm (.) e: duplicate exm into adjacent pairs so in1's
            # innermost AP step is 1 (two REAL f16s per 32-bit read) and the
            # TT multiply can engage the DVE 2x packed mode.
            ex2 = sml.tile([P, H, WCAP, 2], _F16, tag="ex2")
            for h in range(H):
                for j in range(2):
                    nc.vector.tensor_copy(
                        ex2[:, h, :W, j].rearrange("p (s k) -> p s k", s=S),
                        exmm[:, h, :, :kwb])
            G = gp.tile([P, WCAP, F], _F16, tag="G")
            Gv = G[:, :W, :].rearrange("p w (c two) -> p w c two", two=2)
            ev = eS[:, :W, 0:F].rearrange("p w (c two) -> p w c two", two=2)
            for h in range(H):
                nc.vector.tensor_tensor(
                    out=Gv[:, :, h * 32:(h + 1) * 32, :],
                    in0=ev[:, :, h * 32:(h + 1) * 32, :],
                    in1=ex2[:, h, :W, :].unsqueeze(2).to_broadcast(
                        [P, W, 32, 2]),
                    op=mybir.AluOpType.mult)

            # weighted segment-sum on PE: identity-lhsT accumulation
            ngrp = (S + 3) // 4
            for g in range(ngrp):
                s0 = g * 4
                nseg = min(4, S - s0)
                ps4 = ps4p.tile([P, 4 * P], _F32, tag="ps4", space="PSUM")
                for si in range(nseg):
                    s = s0 + si
                    for k in range(kwb):
                        nc.tensor.matmul(
                            ps4[:, si * P:(si + 1) * P],
                            id_sb[:], G[:, s * kwb + k, 0:F],
                            start=(k == 0), stop=(k == kwb - 1),
                            skip_group_check=True)
                hsb = outp.tile([P, 4 * P], _F32, tag="hsb")
                nc.vector.tensor_copy(hsb[:, :nseg * P], ps4[:, :nseg * P])
                t0 = tiles[s0]
                nc.sync.dma_start(
                    hout[t0 * P:(t0 + nseg) * P, :].rearrange(
                        "(s p) c -> p s c", p=P),
                    hsb[:, :nseg * P].rearrange("p (s c) -> p s c", s=nseg))

        NB = len(blocks)
        prod = production(0)
        for bb in range(NB):
            cur = prod
            AB = consume_abs(cur)
            if bb + 1 < NB:
                prod = production(bb + 1)
            consume(cur, AB)
    return nc


def _compile_layer(Kx, plan, npos, stacked):
    nc = bacc.Bacc("TRN2", target_bir_lowering=False, debug=False,
                   enable_asserts=False, num_devices=NCORES,
                   num_swdge_queues=1)
    _build_layer(nc, Kx, plan, npos, stacked)
    nc.compile()
    return nc


# ======================= top-level =======================

def _core_inputs(plan, lc, feats):
    """feats: [Bn] arrays [N, K] float32 (node-id space, std channels)."""
    K = feats[0].shape[1]
    Kx = lc["Kx"]
    stacked = lc["stacked"]
    maps = []
    ftp = []
    for g in range(Bn):
        fp = np.zeros((Kx, N + 1), np.float16)
        fp[:K, :N] = feats[g].T
        if Kx > K:
            fp[K, :N] = 1.0
        ftp.append(fp)
    ident = np.eye(P, dtype=np.float16)
    for core in range(NCORES):
        g, q = core // 4, core % 4
        fg = ftp[g][:, plan["srcid"][q]]
        if stacked:
            fg = np.vstack([fg, ftp[g][:, plan["dstid"][q]]])
            wl = np.vstack([lc["Wl_ext"], lc["Wr_ext"]])
            maps.append({
                "fg": np.ascontiguousarray(fg),
                "Wl_ext": np.ascontiguousarray(wl),
                "mask01": plan["mask01"][q],
                "ident": ident,
            })
        else:
            maps.append({
                "fg": np.ascontiguousarray(fg),
                "Wl_ext": lc["Wl_ext"], "Wr_ext": lc["Wr_ext"],
                "mask01": plan["mask01"][q],
                "ident": ident,
            })
    return maps, ftp


_RESULTS_LOG = {}


def _gather_h(plan, res, lc, ftp):
    """Host: h = (hout/den - xr) * inv + bias, un-permuted."""
    perm_inv = np.empty(F, np.int64)
    perm_inv[lc["perm"]] = np.arange(F)
    # device-matching xr per graph: [N, RW-first-F]
    xr_h = [ftp[g].astype(np.float32).T @ lc["Wr_ext"].astype(np.float32)
            for g in range(Bn)]
    h = np.zeros((Bn, N, F), np.float32)
    for core in range(NCORES):
        g, q = core // 4, core % 4
        rows = res.results[core]["hout"].astype(np.float32)
        dens = res.results[core]["dend"].astype(np.float32)
        own = plan["own_ranks"][q]
        real = own < N
        ids = plan["order"][own[real]]
        r = rows[real]
        d = dens[real]
        dexp = np.repeat(d, C, axis=1)                 # [n, F]
        h[g, ids] = r / dexp - xr_h[g][ids][:, :F]
    h = h * lc["inv"][None, None, :] + lc["bias_p"][None, None, :]
    return h[:, :, perm_inv]


def kernel(x, edge_index, Wl1, bl1, Wr1, br1, att1, bias1,
           Wl2, bl2, Wr2, br2, att2, bias2):
    x = np.asarray(x, np.float32)
    edge_index = np.asarray(edge_index)
    plan = _plan(edge_index)
    lc1 = _layer_consts(Wl1, bl1, Wr1, br1, att1, bias1)
    lc2 = _layer_consts(Wl2, bl2, Wr2, br2, att2, bias2)

    feats1 = [np.ascontiguousarray(x[g].T) for g in range(Bn)]
    nc1 = _compile_layer(lc1["Kx"], plan, lc1["npos"], lc1["stacked"])
    maps1, ftp1 = _core_inputs(plan, lc1, feats1)
    res1 = run_bass_kernel_spmd(nc1, maps1, list(range(NCORES)))
    _RESULTS_LOG["l1"] = res1
    h1 = _gather_h(plan, res1, lc1, ftp1)

    feats2 = [np.ascontiguousarray(h1[g]) for g in range(Bn)]
    nc2 = _compile_layer(lc2["Kx"], plan, lc2["npos"], lc2["stacked"])
    maps2, ftp2 = _core_inputs(plan, lc2, feats2)
    res2 = run_bass_kernel_spmd(nc2, maps2, list(range(NCORES)))
    _RESULTS_LOG["l2"] = res2
    h2 = _gather_h(plan, res2, lc2, ftp2)

    return np.ascontiguousarray(np.transpose(h2, (0, 2, 1)))


# revision 43
# speedup vs baseline: 2.3704x; 1.1653x over previous
"""Trainium2 Bass kernel v4 for 2-layer GATv2 (nn_GCNAttn_1494648619259).

Per-dst-slot layout as v3: dst node = SBUF partition, its in-edges along the
free axis (slot k=0 = self loop); host pre-gathers per-edge SOURCE features
(the halo gather).  v4 restructures the device pipeline:

  * e = y_l[s] + y_r[d] is produced DIRECTLY in PSUM by two accumulating
    matmuls per edge-slab (Wl on the edge column + Wr on the segment's self
    column) - the old eatt DVE pass and xr2 path are gone.  The L-columns
    (F:F+H) hold 0.55*(L[s]+R[d]) for the score's linear part.
  * |e| on GpSimd (tensor_scalar abs_max 0) - frees DVE.
  * score contraction sum_c sign_c*|e_c| via a compile-time sign-range
    halving TREE on DVE f16 (<=2 ops/level/head), not 1x tensor_reduce.
  * softmax un-normalized: weights = exp(score - selfscore) * mask (f16);
    den is written out and the division happens on HOST.
  * weighted segment-sum on the PE: identity-lhsT matmuls accumulate
    G = exm (.) e chunks into a PSUM tile per dst-tile; host subtracts
    den*xr (out = sum exm*e - den*xr = sum exm*y_l) and normalizes.

Sharding: 8 cores = 2 graphs x 4 quarters (unchanged from v3).
"""
import numpy as np
from contextlib import ExitStack

import concourse.bass as bass
import concourse.mybir as mybir
import concourse.tile as tile
from concourse import bacc
from concourse.bass_utils import run_bass_kernel_spmd

# ---- problem constants ----
H = 2
C = 64
F = 2 * C            # 128
NEG = 0.1
A_ = (1 + NEG) / 2.0  # 0.55
B_ = (1 - NEG) / 2.0  # 0.45
N = 20000
Bn = 2
F_IN = 32
NT = 160
P = 128
NPAD = NT * P        # 20480
NG = 40              # groups == own tiles per core
NCORES = 8
RW = 132             # matmul out row: 128 y + 2 (0.55*L) + 2 pad
SLAB = 6             # k-cols per convert batch: 2 PSUM banks, 3 cols each
BANK = 512           # PSUM bank, f32 elems
WCAP = 64            # max k-columns (S*kwb) per processing block

_F32 = mybir.dt.float32
_F16 = mybir.dt.float16


# ======================= host-side planning =======================

def _plan(edge_index):
    src = edge_index[0].astype(np.int64)
    dst = edge_index[1].astype(np.int64)
    E = len(src)

    deg = np.bincount(dst, minlength=N)          # in-degree excl self loop
    order = np.argsort(-deg, kind="stable")
    rank_of = np.empty(N, np.int64)
    rank_of[order] = np.arange(N)

    deg_by_rank = np.zeros(NPAD, np.int64)
    deg_by_rank[:N] = deg[order]
    KW = np.zeros(NG, np.int64)
    for j in range(NG):
        KW[j] = deg_by_rank[j * 512:(j + 1) * 512].max() + 1
    KW = ((KW + 1) // 2) * 2                      # multiple of 2

    # blocks: greedily merge adjacent groups while S*kwb <= WCAP
    blocks = []                                   # list of (tiles, KWB)
    j = 0
    while j < NG:
        kwb = int(KW[j])
        S = 1
        while j + S < NG and (S + 1) * max(kwb, int(KW[j + S])) <= WCAP:
            kwb = max(kwb, int(KW[j + S]))
            S += 1
        blocks.append((list(range(j, j + S)), kwb))
        j += S
    boff = []                                     # slot offset per block
    off = 0
    for tiles, kwb in blocks:
        boff.append(off)
        off += len(tiles) * kwb
    SLOTW = off                                   # total k-columns
    SLOT = SLOTW * P

    # node id per rank; dummy ranks (>= N) -> id N (zero feature column)
    ids = np.concatenate([order, np.full(NPAD - N, N, np.int64)])

    rd = rank_of[dst]
    qd = (rd // P) % 4
    jd = rd // 512
    pd = rd % P
    sort_d = np.argsort(rd, kind="stable")
    starts = np.searchsorted(rd[sort_d], rd)
    invpos = np.empty(E, np.int64)
    invpos[sort_d] = np.arange(E)
    kidx = invpos - starts
    assert np.all(kidx + 1 <= KW[jd] - 1)

    # per-group column offset: group j -> block bb, seg s
    jcol = np.zeros(NG, np.int64)                 # k-col offset of group j
    for (tiles, kwb), off in zip(blocks, boff):
        for s, t in enumerate(tiles):
            jcol[t] = off + s * kwb

    srcid = np.zeros((4, SLOT), np.int32)
    dstid = np.zeros((4, SLOT), np.int32)
    mask01 = np.zeros((4, P, SLOTW), np.float16)
    own_ranks = []
    for q in range(4):
        r_all = np.arange(NPAD)
        own = r_all[(r_all // P) % 4 == q]
        own_ranks.append(own)
        sid = np.empty(SLOT, np.int64)
        mq = np.zeros((P, SLOTW), np.float16)
        for (tiles, kwb), off in zip(blocks, boff):
            for s, t in enumerate(tiles):
                ranks = np.arange(512 * t + 128 * q, 512 * t + 128 * (q + 1))
                dst_ids = ids[ranks]
                co = off + s * kwb
                sid[co * P:(co + kwb) * P] = np.tile(dst_ids, kwb)
                dslot = deg_by_rank[ranks]
                karr = np.arange(kwb)[None, :]
                mq[:, co:co + kwb] = (karr <= dslot[:, None]).astype(
                    np.float16)
        dstid[q] = sid.astype(np.int32)          # pre-scatter: dst id per slot
        sel = qd == q
        col = (jcol[jd[sel]] + kidx[sel] + 1) * P + pd[sel]
        sid[col] = src[sel]
        srcid[q] = sid.astype(np.int32)
        mask01[q] = mq

    return dict(order=order, rank_of=rank_of, KW=KW, blocks=blocks,
                boff=boff, SLOTW=SLOTW, SLOT=SLOT, srcid=srcid, dstid=dstid,
                mask01=mask01, own_ranks=own_ranks)


def _layer_consts(Wl, bl, Wr, br, att, bias):
    att = np.asarray(att, np.float64)
    perm = np.concatenate([
        h * C + np.concatenate([np.nonzero(att[h] >= 0)[0],
                                np.nonzero(att[h] < 0)[0]])
        for h in range(H)]).astype(np.int64)
    npos = np.array([(att[h] >= 0).sum() for h in range(H)], np.int64)
    attp = att.reshape(-1)[perm]
    aab = np.abs(attp)
    Wl = np.asarray(Wl, np.float64)[:, perm]
    Wr = np.asarray(Wr, np.float64)[:, perm]
    bl = np.asarray(bl, np.float64)[perm]
    br = np.asarray(br, np.float64)[perm]
    K = Wl.shape[0]
    has_bias = bool(np.any(bl != 0) or np.any(br != 0))
    if has_bias:
        Kx = K + 1
        assert Kx <= P, "K=128 with nonzero table bias unsupported"
    else:
        Kx = K
    # stacked mode: [fg_src; fg_dst] with [Wl; Wr] -> one matmul per k-col
    stacked = 2 * Kx <= P
    Wl_ext = np.zeros((Kx, RW), np.float32)
    Wr_ext = np.zeros((Kx, RW), np.float32)
    Wl_ext[:K, :F] = Wl * aab[None, :]
    Wr_ext[:K, :F] = Wr * aab[None, :]
    if Kx > K:
        Wl_ext[K, :F] = bl * aab
        Wr_ext[K, :F] = br * aab
    for h in range(H):
        sl = slice(h * C, (h + 1) * C)
        Wl_ext[:K, F + h] = A_ * (Wl[:, sl] @ attp[sl])
        Wr_ext[:K, F + h] = A_ * (Wr[:, sl] @ attp[sl])
        if Kx > K:
            Wl_ext[K, F + h] = A_ * (bl[sl] @ attp[sl])
            Wr_ext[K, F + h] = A_ * (br[sl] @ attp[sl])
    return dict(perm=perm, npos=npos, Kx=Kx, stacked=stacked,
                Wl_ext=Wl_ext.astype(np.float16),
                Wr_ext=Wr_ext.astype(np.float16),
                inv=(1.0 / aab).astype(np.float32),
                bias_p=np.asarray(bias, np.float32)[perm])


def _tree_schedule(p0):
    """Halving-tree ops for one head's 64 channels, tag-monotone.

    Returns (levels, sigma): levels = list of op-lists; each op is
    (i0, i1, sub) - combine cur[i0:i1] with cur[i0+w:i1+w] via sub?lo-hi:lo+hi
    producing out[i0:i1].  Tags stay [+^a, -^(w-a)] with a = min(a, w).
    sigma = final +-1 sign of the single output value.
    """
    a = int(p0)
    w = 64
    levels = []
    while w > 1:
        w //= 2
        ops = []
        hi_a = max(0, min(a - w, w))   # (+,+) range [0, hi_a)
        mid = min(a, w)                # (+,-) range [hi_a, mid)
        if hi_a > 0:
            ops.append((0, hi_a, False))
        if mid > hi_a:
            ops.append((hi_a, mid, True))
        if w > mid:
            ops.append((mid, w, False))   # (-,-) -> add, tag -1
        levels.append(ops)
        a = min(a, w)
    sigma = 1.0 if a >= 1 else -1.0
    return levels, sigma


# ======================= bass program =======================

def _build_layer(nc, Kx, plan, npos, stacked):
    blocks = plan["blocks"]
    boff = plan["boff"]
    SLOTW = plan["SLOTW"]
    SLOT = plan["SLOT"]
    Kf = 2 * Kx if stacked else Kx              # fg rows per k-column
    # NOTE: packing 2 k-cols across 128 partitions (base-64 lhsT matmuls)
    # faults on HW - keep PK=1
    PK = 1
    Kp = PK * Kf

    fg_in = nc.dram_tensor("fg", [Kp, SLOT // PK], _F16,
                           kind="ExternalInput").ap()
    # weights replicated in each PK partition strip (matmul needs equal
    # base_partition for lhsT and rhs)
    Wl_in = nc.dram_tensor("Wl_ext", [Kp, RW], _F16, kind="ExternalInput").ap()
    if not stacked:
        Wr_in = nc.dram_tensor("Wr_ext", [Kx, RW], _F16,
                               kind="ExternalInput").ap()
    mask_in = nc.dram_tensor("mask01", [P, SLOTW], _F16,
                             kind="ExternalInput").ap()
    id_in = nc.dram_tensor("ident", [P, P], _F16, kind="ExternalInput").ap()
    hout = nc.dram_tensor("hout", [NG * P, F], _F32, kind="ExternalOutput").ap()
    dend = nc.dram_tensor("dend", [NG * P, H], _F32, kind="ExternalOutput").ap()

    p0, p1 = int(npos[0]), int(npos[1])
    tree_h = [_tree_schedule(p0), _tree_schedule(p1)]

    with tile.TileContext(nc) as tc, ExitStack() as ctx:
        const = ctx.enter_context(tc.tile_pool(name="const", bufs=1))
        fgp = ctx.enter_context(tc.tile_pool(name="fgp", bufs=2))
        psp = ctx.enter_context(tc.tile_pool(name="psp", bufs=3, space="PSUM"))
        ps4p = ctx.enter_context(tc.tile_pool(name="ps4p", bufs=2,
                                              space="PSUM"))
        esp = ctx.enter_context(tc.tile_pool(name="esp", bufs=3))
        abp = ctx.enter_context(tc.tile_pool(name="abp", bufs=2))
        trp = ctx.enter_context(tc.tile_pool(name="trp", bufs=2))
        gp = ctx.enter_context(tc.tile_pool(name="gp", bufs=2))
        sml = ctx.enter_context(tc.tile_pool(name="sml", bufs=2))
        outp = ctx.enter_context(tc.tile_pool(name="outp", bufs=2))

        wl_sb = const.tile([Kp, RW], _F16)
        nc.sync.dma_start(wl_sb[:], Wl_in[:])
        if not stacked:
            wr_sb = const.tile([Kx, RW], _F16)
            nc.sync.dma_start(wr_sb[:], Wr_in[:])
        mask_sb = const.tile([P, SLOTW], _F16)
        nc.sync.dma_start(mask_sb[:], mask_in[:])
        id_sb = const.tile([P, P], _F16)
        nc.sync.dma_start(id_sb[:], id_in[:])

        def production(bb):
            """fg DMA + accumulating e-matmuls + PSUM->f16 converts."""
            tiles, kwb = blocks[bb]
            off = boff[bb]
            S = len(tiles)
            W = S * kwb
            fgt = fgp.tile([Kp, (WCAP // PK) * P], _F16, tag="fg")
            nc.sync.dma_start(
                fgt[:, :(W // PK) * P],
                fg_in[:, (off // PK) * P:((off + W) // PK) * P])
            eS = esp.tile([P, WCAP, RW], _F16, tag="eS")
            nb = (W + SLAB - 1) // SLAB
            for b in range(nb):
                k0 = b * SLAB
                ns = min(SLAB, W - k0)
                ps = psp.tile([P, 2 * BANK], _F32, tag="ps", space="PSUM")
                for s in range(ns):
                    k = k0 + s
                    po = (s // 3) * BANK + (s % 3) * RW
                    if stacked:
                        rr = (k % PK) * Kf
                        nc.tensor.matmul(
                            ps[:, po:po + RW],
                            fgt[rr:rr + Kf, (k // PK) * P:(k // PK + 1) * P],
                            wl_sb[rr:rr + Kf, :],
                            start=True, stop=True, skip_group_check=True)
                    else:
                        sc_col = (k // kwb) * kwb    # segment self column
                        nc.tensor.matmul(
                            ps[:, po:po + RW],
                            fgt[:, k * P:(k + 1) * P], wl_sb[:],
                            start=True, stop=False, skip_group_check=True)
                        nc.tensor.matmul(
                            ps[:, po:po + RW],
                            fgt[:, sc_col * P:(sc_col + 1) * P], wr_sb[:],
                            start=False, stop=True, skip_group_check=True)
                if ns == SLAB:
                    nc.scalar.activation(
                        eS[:, k0:k0 + ns, :].rearrange(
                            "p (b k) r -> p b (k r)", b=2),
                        ps[:].rearrange(
                            "p (b c) -> p b c", b=2)[:, :, :3 * RW],
                        mybir.ActivationFunctionType.Copy)
                else:
                    for b0 in range(0, ns, 3):
                        n0 = min(3, ns - b0)
                        nc.scalar.activation(
                            eS[:, k0 + b0:k0 + b0 + n0, :].rearrange(
                                "p k r -> p (k r)"),
                            ps[:, (b0 // 3) * BANK:
                               (b0 // 3) * BANK + n0 * RW],
                            mybir.ActivationFunctionType.Copy)
            return (tiles, kwb, S, W, off, eS)

        def consume_abs(prod):
            """|e| ops: only need this block's eS - emitted BEFORE the next
            block's production so they sit ahead of its converts in the ACT
            FIFO (convoy breaker)."""
            tiles, kwb, S, W, off, eS = prod
            # split by measured rates: ~60% ACT Abs, 40% DVE max(-x, x)
            AB = abp.tile([P, WCAP, F], _F16, tag="AB")
            cut = (3 * W // 5 + 1) & ~1
            cut = min(cut, W)
            if cut > 0:
                nc.scalar.activation(
                    AB[:, 0:cut, :], eS[:, 0:cut, 0:F],
                    mybir.ActivationFunctionType.Abs)
            if W > cut:
                # |x| = max(-x, x); abs_max is not a legal TT/TS alu op
                nc.vector.scalar_tensor_tensor(
                    out=AB[:, cut:W, :], in0=eS[:, cut:W, 0:F],
                    scalar=-1.0, in1=eS[:, cut:W, 0:F],
                    op0=mybir.AluOpType.mult, op1=mybir.AluOpType.max)
            return AB

        def consume(prod, AB):
            tiles, kwb, S, W, off, eS = prod
            eS_r = eS[:, :W, :].rearrange("p (s k) r -> p s k r", s=S)

            # sign-range halving tree per head -> T1 [P, W, 2]
            tiers = {}
            cur_w = 32
            Tprev = None
            # level widths: 32,16,8,4,2,1; tile holds both heads side by side
            for li in range(6):
                w = 32 >> li
                T = trp.tile([P, WCAP, 2 * w], _F16, tag=f"T{w}")
                for h in range(H):
                    ops = tree_h[h][0][li]
                    if li == 0:
                        src = AB[:, :W, h * 64:(h + 1) * 64]
                    else:
                        pw = 2 * w
                        src = Tprev[:, :W, h * pw:(h + 1) * pw]
                    dst = T[:, :W, h * w:(h + 1) * w]
                    for (i0, i1, sub) in ops:
                        nc.vector.tensor_tensor(
                            out=dst[:, :, i0:i1],
                            in0=src[:, :, i0:i1],
                            in1=src[:, :, w + i0:w + i1],
                            op=(mybir.AluOpType.subtract if sub
                                else mybir.AluOpType.add))
                Tprev = T
            T1 = Tprev   # [P, W(cap), 2]
            T1_r = T1[:, :W, :].rearrange("p (s k) t -> p t s k", s=S)

            # sc[h] = (sigma_h*B_)*T1[h] + 0.55*(L+R)  (col F+h of eS)
            sc = sml.tile([P, H, S, WCAP], _F32, tag="sc")
            for h in range(H):
                sig = tree_h[h][1]
                nc.vector.scalar_tensor_tensor(
                    out=sc[:, h, :, :kwb], in0=T1_r[:, h],
                    scalar=float(B_ * sig),
                    in1=eS_r[:, :, :, F + h],
                    op0=mybir.AluOpType.mult, op1=mybir.AluOpType.add)

            # shift by the self slot's score (softmax-invariant; keeps the
            # un-normalized f16 weights in range): exp bias per (h, s)
            selfneg = sml.tile([P, H, S], _F32, tag="sn")
            nc.scalar.activation(
                selfneg[:], sc[:, :, :, 0],
                mybir.ActivationFunctionType.Copy, scale=-1.0)
            exm = sml.tile([P, H, S, WCAP], _F16, tag="exm")
            for h in range(H):
                for s in range(S):
                    nc.scalar.activation(
                        exm[:, h, s, :kwb], sc[:, h, s, :kwb],
                        mybir.ActivationFunctionType.Exp,
                        bias=selfneg[:, h, s:s + 1])

            # mask
            exmm = sml.tile([P, H, S, WCAP], _F16, tag="exmm")
            mvv = mask_sb[:, off:off + W].rearrange(
                "p (s k) -> p s k", s=S).unsqueeze(1).to_broadcast(
                [P, H, S, kwb])
            nc.vector.tensor_tensor(
                out=exmm[:, :, :, :kwb], in0=exm[:, :, :, :kwb], in1=mvv,
                op=mybir.AluOpType.mult)

            # den + DMA per segment
            den = sml.tile([P, H, S], _F32, tag="den")
            nc.vector.tensor_reduce(
                out=den[:], in_=exmm[:, :, :, :kwb],
                axis=mybir.AxisListType.X, op=mybir.AluOpType.add)
            for s, t in enumerate(tiles):
                nc.sync.dma_start(dend[t * P:(t + 1) * P, :], den[:, :, s])

            # G = exm (.) e: duplicate exm into adjacent pairs so in1's
            # innermost AP step is 1 (two REAL f16s per 32-bit read) and the
            # TT multiply can engage the DVE 2x packed mode.
            ex2 = sml.tile([P, H, WCAP, 2], _F16, tag="ex2")
            for h in range(H):
                for j in range(2):
                    nc.vector.tensor_copy(
                        ex2[:, h, :W, j].rearrange("p (s k) -> p s k", s=S),
                        exmm[:, h, :, :kwb])
            G = gp.tile([P, WCAP, F], _F16, tag="G")
            Gv = G[:, :W, :].rearrange("p w (c two) -> p w c two", two=2)
            ev = eS[:, :W, 0:F].rearrange("p w (c two) -> p w c two", two=2)
            for h in range(H):
                nc.vector.tensor_tensor(
                    out=Gv[:, :, h * 32:(h + 1) * 32, :],
                    in0=ev[:, :, h * 32:(h + 1) * 32, :],
                    in1=ex2[:, h, :W, :].unsqueeze(2).to_broadcast(
                        [P, W, 32, 2]),
                    op=mybir.AluOpType.mult)

            # weighted segment-sum on PE: identity-lhsT accumulation
            ngrp = (S + 3) // 4
            for g in range(ngrp):
                s0 = g * 4
                nseg = min(4, S - s0)
                ps4 = ps4p.tile([P, 4 * P], _F32, tag="ps4", space="PSUM")
                for si in range(nseg):
                    s = s0 + si
                    for k in range(kwb):
                        nc.tensor.matmul(
                            ps4[:, si * P:(si + 1) * P],
                            id_sb[:], G[:, s * kwb + k, 0:F],
                            start=(k == 0), stop=(k == kwb - 1),
                            skip_group_check=True)
                hsb = outp.tile([P, 4 * P], _F32, tag="hsb")
                nc.vector.tensor_copy(hsb[:, :nseg * P], ps4[:, :nseg * P])
                t0 = tiles[s0]
                nc.sync.dma_start(
                    hout[t0 * P:(t0 + nseg) * P, :].rearrange(
                        "(s p) c -> p s c", p=P),
                    hsb[:, :nseg * P].rearrange("p (s c) -> p s c", s=nseg))

        NB = len(blocks)
        prod = production(0)
        for bb in range(NB):
            cur = prod
            AB = consume_abs(cur)
            if bb + 1 < NB:
                prod = production(bb + 1)
            consume(cur, AB)
    return nc


def _compile_layer(Kx, plan, npos, stacked):
    nc = bacc.Bacc("TRN2", target_bir_lowering=False, debug=False,
                   enable_asserts=False, num_devices=NCORES,
                   num_swdge_queues=1)
    _build_layer(nc, Kx, plan, npos, stacked)
    nc.compile()
    return nc


# ======================= top-level =======================

def _core_inputs(plan, lc, feats):
    """feats: [Bn] arrays [N, K] float32 (node-id space, std channels)."""
    K = feats[0].shape[1]
    Kx = lc["Kx"]
    stacked = lc["stacked"]
    maps = []
    ftp = []
    for g in range(Bn):
        fp = np.zeros((Kx, N + 1), np.float16)
        fp[:K, :N] = feats[g].T
        if Kx > K:
            fp[K, :N] = 1.0
        ftp.append(fp)
    ident = np.eye(P, dtype=np.float16)
    for core in range(NCORES):
        g, q = core // 4, core % 4
        fg = ftp[g][:, plan["srcid"][q]]
        if stacked:
            fg = np.vstack([fg, ftp[g][:, plan["dstid"][q]]])
            wl = np.vstack([lc["Wl_ext"], lc["Wr_ext"]])
            maps.append({
                "fg": np.ascontiguousarray(fg),
                "Wl_ext": np.ascontiguousarray(wl),
                "mask01": plan["mask01"][q],
                "ident": ident,
            })
        else:
            maps.append({
                "fg": np.ascontiguousarray(fg),
                "Wl_ext": lc["Wl_ext"], "Wr_ext": lc["Wr_ext"],
                "mask01": plan["mask01"][q],
                "ident": ident,
            })
    return maps, ftp


_RESULTS_LOG = {}


def _gather_h(plan, res, lc, ftp):
    """Host: h = (hout/den - xr) * inv + bias, un-permuted."""
    perm_inv = np.empty(F, np.int64)
    perm_inv[lc["perm"]] = np.arange(F)
    # device-matching xr per graph: [N, RW-first-F]
    xr_h = [ftp[g].astype(np.float32).T @ lc["Wr_ext"].astype(np.float32)
            for g in range(Bn)]
    h = np.zeros((Bn, N, F), np.float32)
    for core in range(NCORES):
        g, q = core // 4, core % 4
        rows = res.results[core]["hout"].astype(np.float32)
        dens = res.results[core]["dend"].astype(np.float32)
        own = plan["own_ranks"][q]
        real = own < N
        ids = plan["order"][own[real]]
        r = rows[real]
        d = dens[real]
        dexp = np.repeat(d, C, axis=1)                 # [n, F]
        h[g, ids] = r / dexp - xr_h[g][ids][:, :F]
    h = h * lc["inv"][None, None, :] + lc["bias_p"][None, None, :]
    return h[:, :, perm_inv]


def kernel(x, edge_index, Wl1, bl1, Wr1, br1, att1, bias1,
           Wl2, bl2, Wr2, br2, att2, bias2):
    x = np.asarray(x, np.float32)
    edge_index = np.asarray(edge_index)
    plan = _plan(edge_index)
    lc1 = _layer_consts(Wl1, bl1, Wr1, br1, att1, bias1)
    lc2 = _layer_consts(Wl2, bl2, Wr2, br2, att2, bias2)

    feats1 = [np.ascontiguousarray(x[g].T) for g in range(Bn)]
    nc1 = _compile_layer(lc1["Kx"], plan, lc1["npos"], lc1["stacked"])
    maps1, ftp1 = _core_inputs(plan, lc1, feats1)
    res1 = run_bass_kernel_spmd(nc1, maps1, list(range(NCORES)))
    _RESULTS_LOG["l1"] = res1
    h1 = _gather_h(plan, res1, lc1, ftp1)

    feats2 = [np.ascontiguousarray(h1[g]) for g in range(Bn)]
    nc2 = _compile_layer(lc2["Kx"], plan, lc2["npos"], lc2["stacked"])
    maps2, ftp2 = _core_inputs(plan, lc2, feats2)
    res2 = run_bass_kernel_spmd(nc2, maps2, list(range(NCORES)))
    _RESULTS_LOG["l2"] = res2
    h2 = _gather_h(plan, res2, lc2, ftp2)

    return np.ascontiguousarray(np.transpose(h2, (0, 2, 1)))


# revision 45
# speedup vs baseline: 2.3988x; 1.0120x over previous
"""Trainium2 Bass kernel v4 for 2-layer GATv2 (nn_GCNAttn_1494648619259).

Per-dst-slot layout as v3: dst node = SBUF partition, its in-edges along the
free axis (slot k=0 = self loop); host pre-gathers per-edge SOURCE features
(the halo gather).  v4 restructures the device pipeline:

  * e = y_l[s] + y_r[d] is produced DIRECTLY in PSUM by two accumulating
    matmuls per edge-slab (Wl on the edge column + Wr on the segment's self
    column) - the old eatt DVE pass and xr2 path are gone.  The L-columns
    (F:F+H) hold 0.55*(L[s]+R[d]) for the score's linear part.
  * |e| on GpSimd (tensor_scalar abs_max 0) - frees DVE.
  * score contraction sum_c sign_c*|e_c| via a compile-time sign-range
    halving TREE on DVE f16 (<=2 ops/level/head), not 1x tensor_reduce.
  * softmax un-normalized: weights = exp(score - selfscore) * mask (f16);
    den is written out and the division happens on HOST.
  * weighted segment-sum on the PE: identity-lhsT matmuls accumulate
    G = exm (.) e chunks into a PSUM tile per dst-tile; host subtracts
    den*xr (out = sum exm*e - den*xr = sum exm*y_l) and normalizes.

Sharding: 8 cores = 2 graphs x 4 quarters (unchanged from v3).
"""
import numpy as np
from contextlib import ExitStack

import concourse.bass as bass
import concourse.mybir as mybir
import concourse.tile as tile
from concourse import bacc
from concourse.bass_utils import run_bass_kernel_spmd

# ---- problem constants ----
H = 2
C = 64
F = 2 * C            # 128
NEG = 0.1
A_ = (1 + NEG) / 2.0  # 0.55
B_ = (1 - NEG) / 2.0  # 0.45
N = 20000
Bn = 2
F_IN = 32
NT = 160
P = 128
NPAD = NT * P        # 20480
NG = 40              # groups == own tiles per core
NCORES = 8
RW = 132             # matmul out row: 128 y + 2 (0.55*L) + 2 pad
SLAB = 6             # k-cols per convert batch: 2 PSUM banks, 3 cols each
BANK = 512           # PSUM bank, f32 elems
WCAP = 64            # max k-columns (S*kwb) per processing block

_F32 = mybir.dt.float32
_F16 = mybir.dt.float16


# ======================= host-side planning =======================

def _plan(edge_index):
    src = edge_index[0].astype(np.int64)
    dst = edge_index[1].astype(np.int64)
    E = len(src)

    deg = np.bincount(dst, minlength=N)          # in-degree excl self loop
    order = np.argsort(-deg, kind="stable")
    rank_of = np.empty(N, np.int64)
    rank_of[order] = np.arange(N)

    deg_by_rank = np.zeros(NPAD, np.int64)
    deg_by_rank[:N] = deg[order]
    KW = np.zeros(NG, np.int64)
    for j in range(NG):
        KW[j] = deg_by_rank[j * 512:(j + 1) * 512].max() + 1
    KW = ((KW + 1) // 2) * 2                      # multiple of 2

    # blocks: greedily merge adjacent groups while S*kwb <= WCAP
    blocks = []                                   # list of (tiles, KWB)
    j = 0
    while j < NG:
        kwb = int(KW[j])
        S = 1
        while j + S < NG and (S + 1) * max(kwb, int(KW[j + S])) <= WCAP:
            kwb = max(kwb, int(KW[j + S]))
            S += 1
        blocks.append((list(range(j, j + S)), kwb))
        j += S
    boff = []                                     # slot offset per block
    off = 0
    for tiles, kwb in blocks:
        boff.append(off)
        off += len(tiles) * kwb
    SLOTW = off                                   # total k-columns
    SLOT = SLOTW * P

    # node id per rank; dummy ranks (>= N) -> id N (zero feature column)
    ids = np.concatenate([order, np.full(NPAD - N, N, np.int64)])

    rd = rank_of[dst]
    qd = (rd // P) % 4
    jd = rd // 512
    pd = rd % P
    sort_d = np.argsort(rd, kind="stable")
    starts = np.searchsorted(rd[sort_d], rd)
    invpos = np.empty(E, np.int64)
    invpos[sort_d] = np.arange(E)
    kidx = invpos - starts
    assert np.all(kidx + 1 <= KW[jd] - 1)

    # per-group column offset: group j -> block bb, seg s
    jcol = np.zeros(NG, np.int64)                 # k-col offset of group j
    for (tiles, kwb), off in zip(blocks, boff):
        for s, t in enumerate(tiles):
            jcol[t] = off + s * kwb

    srcid = np.zeros((4, SLOT), np.int32)
    dstid = np.zeros((4, SLOT), np.int32)
    mask01 = np.zeros((4, P, SLOTW), np.float16)
    own_ranks = []
    for q in range(4):
        r_all = np.arange(NPAD)
        own = r_all[(r_all // P) % 4 == q]
        own_ranks.append(own)
        sid = np.empty(SLOT, np.int64)
        mq = np.zeros((P, SLOTW), np.float16)
        for (tiles, kwb), off in zip(blocks, boff):
            for s, t in enumerate(tiles):
                ranks = np.arange(512 * t + 128 * q, 512 * t + 128 * (q + 1))
                dst_ids = ids[ranks]
                co = off + s * kwb
                sid[co * P:(co + kwb) * P] = np.tile(dst_ids, kwb)
                dslot = deg_by_rank[ranks]
                karr = np.arange(kwb)[None, :]
                mq[:, co:co + kwb] = (karr <= dslot[:, None]).astype(
                    np.float16)
        dstid[q] = sid.astype(np.int32)          # pre-scatter: dst id per slot
        sel = qd == q
        col = (jcol[jd[sel]] + kidx[sel] + 1) * P + pd[sel]
        sid[col] = src[sel]
        srcid[q] = sid.astype(np.int32)
        mask01[q] = mq

    return dict(order=order, rank_of=rank_of, KW=KW, blocks=blocks,
                boff=boff, SLOTW=SLOTW, SLOT=SLOT, srcid=srcid, dstid=dstid,
                mask01=mask01, own_ranks=own_ranks)


def _layer_consts(Wl, bl, Wr, br, att, bias):
    att = np.asarray(att, np.float64)
    perm = np.concatenate([
        h * C + np.concatenate([np.nonzero(att[h] >= 0)[0],
                                np.nonzero(att[h] < 0)[0]])
        for h in range(H)]).astype(np.int64)
    npos = np.array([(att[h] >= 0).sum() for h in range(H)], np.int64)
    attp = att.reshape(-1)[perm]
    aab = np.abs(attp)
    Wl = np.asarray(Wl, np.float64)[:, perm]
    Wr = np.asarray(Wr, np.float64)[:, perm]
    bl = np.asarray(bl, np.float64)[perm]
    br = np.asarray(br, np.float64)[perm]
    K = Wl.shape[0]
    has_bias = bool(np.any(bl != 0) or np.any(br != 0))
    if has_bias:
        Kx = K + 1
        assert Kx <= P, "K=128 with nonzero table bias unsupported"
    else:
        Kx = K
    # stacked mode: [fg_src; fg_dst] with [Wl; Wr] -> one matmul per k-col
    stacked = 2 * Kx <= P
    Wl_ext = np.zeros((Kx, RW), np.float32)
    Wr_ext = np.zeros((Kx, RW), np.float32)
    Wl_ext[:K, :F] = Wl * aab[None, :]
    Wr_ext[:K, :F] = Wr * aab[None, :]
    if Kx > K:
        Wl_ext[K, :F] = bl * aab
        Wr_ext[K, :F] = br * aab
    for h in range(H):
        sl = slice(h * C, (h + 1) * C)
        Wl_ext[:K, F + h] = A_ * (Wl[:, sl] @ attp[sl])
        Wr_ext[:K, F + h] = A_ * (Wr[:, sl] @ attp[sl])
        if Kx > K:
            Wl_ext[K, F + h] = A_ * (bl[sl] @ attp[sl])
            Wr_ext[K, F + h] = A_ * (br[sl] @ attp[sl])
    return dict(perm=perm, npos=npos, Kx=Kx, stacked=stacked,
                Wl_ext=Wl_ext.astype(np.float16),
                Wr_ext=Wr_ext.astype(np.float16),
                inv=(1.0 / aab).astype(np.float32),
                bias_p=np.asarray(bias, np.float32)[perm])


def _tree_schedule(p0):
    """Halving-tree ops for one head's 64 channels, tag-monotone.

    Returns (levels, sigma): levels = list of op-lists; each op is
    (i0, i1, sub) - combine cur[i0:i1] with cur[i0+w:i1+w] via sub?lo-hi:lo+hi
    producing out[i0:i1].  Tags stay [+^a, -^(w-a)] with a = min(a, w).
    sigma = final +-1 sign of the single output value.
    """
    a = int(p0)
    w = 64
    levels = []
    while w > 1:
        w //= 2
        ops = []
        hi_a = max(0, min(a - w, w))   # (+,+) range [0, hi_a)
        mid = min(a, w)                # (+,-) range [hi_a, mid)
        if hi_a > 0:
            ops.append((0, hi_a, False))
        if mid > hi_a:
            ops.append((hi_a, mid, True))
        if w > mid:
            ops.append((mid, w, False))   # (-,-) -> add, tag -1
        levels.append(ops)
        a = min(a, w)
    sigma = 1.0 if a >= 1 else -1.0
    return levels, sigma


# ======================= bass program =======================

def _build_layer(nc, Kx, plan, npos, stacked):
    blocks = plan["blocks"]
    boff = plan["boff"]
    SLOTW = plan["SLOTW"]
    SLOT = plan["SLOT"]
    Kf = 2 * Kx if stacked else Kx              # fg rows per k-column
    # NOTE: packing 2 k-cols across 128 partitions (base-64 lhsT matmuls)
    # faults on HW - keep PK=1
    PK = 1
    Kp = PK * Kf

    fg_in = nc.dram_tensor("fg", [Kp, SLOT // PK], _F16,
                           kind="ExternalInput").ap()
    # weights replicated in each PK partition strip (matmul needs equal
    # base_partition for lhsT and rhs)
    Wl_in = nc.dram_tensor("Wl_ext", [Kp, RW], _F16, kind="ExternalInput").ap()
    if not stacked:
        Wr_in = nc.dram_tensor("Wr_ext", [Kx, RW], _F16,
                               kind="ExternalInput").ap()
    mask_in = nc.dram_tensor("mask01", [P, SLOTW], _F16,
                             kind="ExternalInput").ap()
    id_in = nc.dram_tensor("ident", [P, P], _F16, kind="ExternalInput").ap()
    hout = nc.dram_tensor("hout", [NG * P, F], _F32, kind="ExternalOutput").ap()
    dend = nc.dram_tensor("dend", [NG * P, H], _F32, kind="ExternalOutput").ap()

    p0, p1 = int(npos[0]), int(npos[1])
    tree_h = [_tree_schedule(p0), _tree_schedule(p1)]

    with tile.TileContext(nc) as tc, ExitStack() as ctx:
        const = ctx.enter_context(tc.tile_pool(name="const", bufs=1))
        fgp = ctx.enter_context(tc.tile_pool(name="fgp", bufs=2))
        psp = ctx.enter_context(tc.tile_pool(name="psp", bufs=3, space="PSUM"))
        ps4p = ctx.enter_context(tc.tile_pool(name="ps4p", bufs=2,
                                              space="PSUM"))
        esp = ctx.enter_context(tc.tile_pool(name="esp", bufs=3))
        abp = ctx.enter_context(tc.tile_pool(name="abp", bufs=2))
        trp = ctx.enter_context(tc.tile_pool(name="trp", bufs=2))
        gp = ctx.enter_context(tc.tile_pool(name="gp", bufs=2))
        sml = ctx.enter_context(tc.tile_pool(name="sml", bufs=2))
        outp = ctx.enter_context(tc.tile_pool(name="outp", bufs=2))

        wl_sb = const.tile([Kp, RW], _F16)
        nc.sync.dma_start(wl_sb[:], Wl_in[:])
        if not stacked:
            wr_sb = const.tile([Kx, RW], _F16)
            nc.sync.dma_start(wr_sb[:], Wr_in[:])
        mask_sb = const.tile([P, SLOTW], _F16)
        nc.sync.dma_start(mask_sb[:], mask_in[:])
        id_sb = const.tile([P, P], _F16)
        nc.sync.dma_start(id_sb[:], id_in[:])

        def production(bb):
            """fg DMA + accumulating e-matmuls + PSUM->f16 converts."""
            tiles, kwb = blocks[bb]
            off = boff[bb]
            S = len(tiles)
            W = S * kwb
            fgt = fgp.tile([Kp, (WCAP // PK) * P], _F16, tag="fg")
            nc.sync.dma_start(
                fgt[:, :(W // PK) * P],
                fg_in[:, (off // PK) * P:((off + W) // PK) * P])
            eS = esp.tile([P, WCAP, RW], _F16, tag="eS")
            nb = (W + SLAB - 1) // SLAB
            for b in range(nb):
                k0 = b * SLAB
                ns = min(SLAB, W - k0)
                ps = psp.tile([P, 2 * BANK], _F32, tag="ps", space="PSUM")
                for s in range(ns):
                    k = k0 + s
                    po = (s // 3) * BANK + (s % 3) * RW
                    if stacked:
                        rr = (k % PK) * Kf
                        nc.tensor.matmul(
                            ps[:, po:po + RW],
                            fgt[rr:rr + Kf, (k // PK) * P:(k // PK + 1) * P],
                            wl_sb[rr:rr + Kf, :],
                            start=True, stop=True, skip_group_check=True)
                    else:
                        sc_col = (k // kwb) * kwb    # segment self column
                        nc.tensor.matmul(
                            ps[:, po:po + RW],
                            fgt[:, k * P:(k + 1) * P], wl_sb[:],
                            start=True, stop=False, skip_group_check=True)
                        nc.tensor.matmul(
                            ps[:, po:po + RW],
                            fgt[:, sc_col * P:(sc_col + 1) * P], wr_sb[:],
                            start=False, stop=True, skip_group_check=True)
                if ns == SLAB:
                    nc.scalar.activation(
                        eS[:, k0:k0 + ns, :].rearrange(
                            "p (b k) r -> p b (k r)", b=2),
                        ps[:].rearrange(
                            "p (b c) -> p b c", b=2)[:, :, :3 * RW],
                        mybir.ActivationFunctionType.Copy)
                else:
                    for b0 in range(0, ns, 3):
                        n0 = min(3, ns - b0)
                        nc.scalar.activation(
                            eS[:, k0 + b0:k0 + b0 + n0, :].rearrange(
                                "p k r -> p (k r)"),
                            ps[:, (b0 // 3) * BANK:
                               (b0 // 3) * BANK + n0 * RW],
                            mybir.ActivationFunctionType.Copy)
            return (tiles, kwb, S, W, off, eS)

        def consume_abs(prod):
            """|e| ops: only need this block's eS - emitted BEFORE the next
            block's production so they sit ahead of its converts in the ACT
            FIFO (convoy breaker)."""
            tiles, kwb, S, W, off, eS = prod
            # split by measured rates: ~60% ACT Abs, 40% DVE max(-x, x)
            AB = abp.tile([P, WCAP, F], _F16, tag="AB")
            cut = (3 * W // 5 + 1) & ~1
            cut = min(cut, W)
            if cut > 0:
                nc.scalar.activation(
                    AB[:, 0:cut, :], eS[:, 0:cut, 0:F],
                    mybir.ActivationFunctionType.Abs)
            if W > cut:
                # |x| = max(-x, x); abs_max is not a legal TT/TS alu op
                nc.vector.scalar_tensor_tensor(
                    out=AB[:, cut:W, :], in0=eS[:, cut:W, 0:F],
                    scalar=-1.0, in1=eS[:, cut:W, 0:F],
                    op0=mybir.AluOpType.mult, op1=mybir.AluOpType.max)
            return AB

        def consume(prod, AB):
            tiles, kwb, S, W, off, eS = prod
            eS_r = eS[:, :W, :].rearrange("p (s k) r -> p s k r", s=S)

            # sign-range halving tree per head -> T1 [P, W, 2]
            tiers = {}
            cur_w = 32
            Tprev = None
            # level widths: 32,16,8,4,2,1; tile holds both heads side by side
            for li in range(6):
                w = 32 >> li
                T = trp.tile([P, WCAP, 2 * w], _F16, tag=f"T{w}")
                for h in range(H):
                    ops = tree_h[h][0][li]
                    if li == 0:
                        src = AB[:, :W, h * 64:(h + 1) * 64]
                    else:
                        pw = 2 * w
                        src = Tprev[:, :W, h * pw:(h + 1) * pw]
                    dst = T[:, :W, h * w:(h + 1) * w]
                    for (i0, i1, sub) in ops:
                        nc.vector.tensor_tensor(
                            out=dst[:, :, i0:i1],
                            in0=src[:, :, i0:i1],
                            in1=src[:, :, w + i0:w + i1],
                            op=(mybir.AluOpType.subtract if sub
                                else mybir.AluOpType.add))
                Tprev = T
            T1 = Tprev   # [P, W(cap), 2]
            T1_r = T1[:, :W, :].rearrange("p (s k) t -> p t s k", s=S)

            # sc[h] = (sigma_h*B_)*T1[h] + 0.55*(L+R)  (col F+h of eS)
            sc = sml.tile([P, H, S, WCAP], _F32, tag="sc")
            for h in range(H):
                sig = tree_h[h][1]
                nc.vector.scalar_tensor_tensor(
                    out=sc[:, h, :, :kwb], in0=T1_r[:, h],
                    scalar=float(B_ * sig),
                    in1=eS_r[:, :, :, F + h],
                    op0=mybir.AluOpType.mult, op1=mybir.AluOpType.add)

            # shift by the self slot's score (softmax-invariant; keeps the
            # un-normalized f16 weights in range): exp bias per (h, s)
            selfneg = sml.tile([P, H, S], _F32, tag="sn")
            nc.scalar.activation(
                selfneg[:], sc[:, :, :, 0],
                mybir.ActivationFunctionType.Copy, scale=-1.0)
            exm = sml.tile([P, H, S, WCAP], _F16, tag="exm")
            for h in range(H):
                for s in range(S):
                    nc.scalar.activation(
                        exm[:, h, s, :kwb], sc[:, h, s, :kwb],
                        mybir.ActivationFunctionType.Exp,
                        bias=selfneg[:, h, s:s + 1])

            # mask
            exmm = sml.tile([P, H, S, WCAP], _F16, tag="exmm")
            mvv = mask_sb[:, off:off + W].rearrange(
                "p (s k) -> p s k", s=S).unsqueeze(1).to_broadcast(
                [P, H, S, kwb])
            nc.vector.tensor_tensor(
                out=exmm[:, :, :, :kwb], in0=exm[:, :, :, :kwb], in1=mvv,
                op=mybir.AluOpType.mult)

            # den + DMA per segment
            den = sml.tile([P, H, S], _F32, tag="den")
            nc.vector.tensor_reduce(
                out=den[:], in_=exmm[:, :, :, :kwb],
                axis=mybir.AxisListType.X, op=mybir.AluOpType.add)
            for s, t in enumerate(tiles):
                nc.sync.dma_start(dend[t * P:(t + 1) * P, :], den[:, :, s])

            # G = exm (.) e: duplicate exm into adjacent pairs so in1's
            # innermost AP step is 1 (two REAL f16s per 32-bit read) and the
            # TT multiply can engage the DVE 2x packed mode.
            ex2 = sml.tile([P, H, WCAP, 2], _F16, tag="ex2")
            for h in range(H):
                for j in range(2):
                    nc.vector.tensor_copy(
                        ex2[:, h, :W, j].rearrange("p (s k) -> p s k", s=S),
                        exmm[:, h, :, :kwb])
            G = gp.tile([P, WCAP, F], _F16, tag="G")
            Gv = G[:, :W, :].rearrange("p w (c two) -> p w c two", two=2)
            ev = eS[:, :W, 0:F].rearrange("p w (c two) -> p w c two", two=2)
            for h in range(H):
                nc.vector.tensor_tensor(
                    out=Gv[:, :, h * 32:(h + 1) * 32, :],
                    in0=ev[:, :, h * 32:(h + 1) * 32, :],
                    in1=ex2[:, h, :W, :].unsqueeze(2).to_broadcast(
                        [P, W, 32, 2]),
                    op=mybir.AluOpType.mult)

            # weighted segment-sum on PE: identity-lhsT accumulation.
            # The PSUM->SBUF copies + hout DMA are DEFERRED to the next
            # block (returned as pending) so they don't stall the DVE FIFO
            # while m4 accumulates.
            pend = []
            ngrp = (S + 3) // 4
            for g in range(ngrp):
                s0 = g * 4
                nseg = min(4, S - s0)
                ps4 = ps4p.tile([P, 4 * P], _F32, tag="ps4", space="PSUM")
                for si in range(nseg):
                    s = s0 + si
                    for k in range(kwb):
                        nc.tensor.matmul(
                            ps4[:, si * P:(si + 1) * P],
                            id_sb[:], G[:, s * kwb + k, 0:F],
                            start=(k == 0), stop=(k == kwb - 1),
                            skip_group_check=True)
                pend.append((ps4, nseg, tiles[s0]))
            return pend

        def flush_out(pend):
            for ps4, nseg, t0 in pend:
                hsb = outp.tile([P, 4 * P], _F32, tag="hsb")
                nc.vector.tensor_copy(hsb[:, :nseg * P], ps4[:, :nseg * P])
                nc.sync.dma_start(
                    hout[t0 * P:(t0 + nseg) * P, :].rearrange(
                        "(s p) c -> p s c", p=P),
                    hsb[:, :nseg * P].rearrange("p (s c) -> p s c", s=nseg))

        NB = len(blocks)
        prod = production(0)
        pend = None
        for bb in range(NB):
            cur = prod
            AB = consume_abs(cur)
            if pend:
                flush_out(pend)
            if bb + 1 < NB:
                prod = production(bb + 1)
            pend = consume(cur, AB)
        flush_out(pend)
    return nc


def _compile_layer(Kx, plan, npos, stacked):
    nc = bacc.Bacc("TRN2", target_bir_lowering=False, debug=False,
                   enable_asserts=False, num_devices=NCORES,
                   num_swdge_queues=1)
    _build_layer(nc, Kx, plan, npos, stacked)
    nc.compile()
    return nc


# ======================= top-level =======================

def _core_inputs(plan, lc, feats):
    """feats: [Bn] arrays [N, K] float32 (node-id space, std channels)."""
    K = feats[0].shape[1]
    Kx = lc["Kx"]
    stacked = lc["stacked"]
    maps = []
    ftp = []
    for g in range(Bn):
        fp = np.zeros((Kx, N + 1), np.float16)
        fp[:K, :N] = feats[g].T
        if Kx > K:
            fp[K, :N] = 1.0
        ftp.append(fp)
    ident = np.eye(P, dtype=np.float16)
    for core in range(NCORES):
        g, q = core // 4, core % 4
        fg = ftp[g][:, plan["srcid"][q]]
        if stacked:
            fg = np.vstack([fg, ftp[g][:, plan["dstid"][q]]])
            wl = np.vstack([lc["Wl_ext"], lc["Wr_ext"]])
            maps.append({
                "fg": np.ascontiguousarray(fg),
                "Wl_ext": np.ascontiguousarray(wl),
                "mask01": plan["mask01"][q],
                "ident": ident,
            })
        else:
            maps.append({
                "fg": np.ascontiguousarray(fg),
                "Wl_ext": lc["Wl_ext"], "Wr_ext": lc["Wr_ext"],
                "mask01": plan["mask01"][q],
                "ident": ident,
            })
    return maps, ftp


_RESULTS_LOG = {}


def _gather_h(plan, res, lc, ftp):
    """Host: h = (hout/den - xr) * inv + bias, un-permuted."""
    perm_inv = np.empty(F, np.int64)
    perm_inv[lc["perm"]] = np.arange(F)
    # device-matching xr per graph: [N, RW-first-F]
    xr_h = [ftp[g].astype(np.float32).T @ lc["Wr_ext"].astype(np.float32)
            for g in range(Bn)]
    h = np.zeros((Bn, N, F), np.float32)
    for core in range(NCORES):
        g, q = core // 4, core % 4
        rows = res.results[core]["hout"].astype(np.float32)
        dens = res.results[core]["dend"].astype(np.float32)
        own = plan["own_ranks"][q]
        real = own < N
        ids = plan["order"][own[real]]
        r = rows[real]
        d = dens[real]
        dexp = np.repeat(d, C, axis=1)                 # [n, F]
        h[g, ids] = r / dexp - xr_h[g][ids][:, :F]
    h = h * lc["inv"][None, None, :] + lc["bias_p"][None, None, :]
    return h[:, :, perm_inv]


def kernel(x, edge_index, Wl1, bl1, Wr1, br1, att1, bias1,
           Wl2, bl2, Wr2, br2, att2, bias2):
    x = np.asarray(x, np.float32)
    edge_index = np.asarray(edge_index)
    plan = _plan(edge_index)
    lc1 = _layer_consts(Wl1, bl1, Wr1, br1, att1, bias1)
    lc2 = _layer_consts(Wl2, bl2, Wr2, br2, att2, bias2)

    feats1 = [np.ascontiguousarray(x[g].T) for g in range(Bn)]
    nc1 = _compile_layer(lc1["Kx"], plan, lc1["npos"], lc1["stacked"])
    maps1, ftp1 = _core_inputs(plan, lc1, feats1)
    res1 = run_bass_kernel_spmd(nc1, maps1, list(range(NCORES)))
    _RESULTS_LOG["l1"] = res1
    h1 = _gather_h(plan, res1, lc1, ftp1)

    feats2 = [np.ascontiguousarray(h1[g]) for g in range(Bn)]
    nc2 = _compile_layer(lc2["Kx"], plan, lc2["npos"], lc2["stacked"])
    maps2, ftp2 = _core_inputs(plan, lc2, feats2)
    res2 = run_bass_kernel_spmd(nc2, maps2, list(range(NCORES)))
    _RESULTS_LOG["l2"] = res2
    h2 = _gather_h(plan, res2, lc2, ftp2)

    return np.ascontiguousarray(np.transpose(h2, (0, 2, 1)))
